# revision 3
# baseline (speedup 1.0000x reference)
"""Multi-head causal attention (B=4, T=2048, C=1024, H=16) on 8 trn2 cores.

Sharding: data-parallel over batch (4) x sequence-parallel over causal query
blocks (2), zig-zag balanced: core = 2*b + half; half 0 gets query blocks
[0,2,4,6,9,11,13,15], half 1 gets [1,3,5,7,8,10,12,14]. Slot s processes
key blocks 0..2s+1; causal boundary via per-core input masks.

Kernel structure (v2): software-pipelined over head pairs c=0..7.
  tick: [Q/K proj pair c+1] x [scores+exp+mask pair c] x [attnv pair c-1]
Scores are row-tiled (K=64, two heads in PE row strips 0/64 concurrently).
attnv is v-stationary (v tile [128,65] incl ones column -> softmax denom in
psum row 64), streaming wide query chunks; output lands as O^T = aT layout
(no transpose phase). Normalization: reciprocal of denom row + rank-2
broadcast matmul (hm2) + partition-shifted DVE multiplies into aT.
"""

import numpy as np
import ml_dtypes

import concourse.bass as bass
import concourse.mybir as mybir
import concourse.tile as tile
from concourse import bacc
from concourse.bass import ts
from concourse.bass_utils import run_bass_kernel_spmd

B, T, C, H, DK = 4, 2048, 1024, 16, 64
P = 128
NB = T // P          # 16 key blocks
SLOTS = 8            # query blocks per core
CB = C // P          # 8 channel blocks = head pairs
SCALE = 1.0 / np.sqrt(DK)
BF16 = mybir.dt.bfloat16
F32 = mybir.dt.float32
EXP = mybir.ActivationFunctionType.Exp

QBLKS = [
    [0, 2, 4, 6, 9, 11, 13, 15],
    [1, 3, 5, 7, 8, 10, 12, 14],
]

# interleaved jb order: evens ACT load per group and matches attnv's
# half-1 read order so expS slot releases track allocations
ILV = [0, 8, 1, 9, 2, 10, 3, 11, 4, 12, 5, 13, 6, 14, 7, 15]

_cache = {}


def _pop(gen, n):
    for _ in range(n):
        try:
            next(gen)()
        except StopIteration:
            return


def _build():
    nc = bacc.Bacc("TRN2", target_bir_lowering=False, debug=False)

    xT = nc.dram_tensor("xT", [C, T], BF16, kind="ExternalInput").ap()
    xTq = nc.dram_tensor("xTq", [C, SLOTS * P], BF16, kind="ExternalInput").ap()
    wqT = nc.dram_tensor("wqT", [C, C], BF16, kind="ExternalInput").ap()
    wkT = nc.dram_tensor("wkT", [C, C], BF16, kind="ExternalInput").ap()
    wvT = nc.dram_tensor("wvT", [C, C], BF16, kind="ExternalInput").ap()
    woT = nc.dram_tensor("woT", [C, C], BF16, kind="ExternalInput").ap()
    bq = nc.dram_tensor("bq", [P, CB], F32, kind="ExternalInput").ap()
    bk = nc.dram_tensor("bk", [P, CB], F32, kind="ExternalInput").ap()
    masks = nc.dram_tensor("masks", [SLOTS, 2, P, P], BF16, kind="ExternalInput").ap()
    hm2 = nc.dram_tensor("hm2", [1, 2, P], BF16, kind="ExternalInput").ap()
    y = nc.dram_tensor("y", [SLOTS * P, C], F32, kind="ExternalOutput").ap()

    def qchunks(jb):
        """score q-chunks for key block jb: [q0,512) and/or [512,1024)."""
        q0 = P * (jb // 2)
        out = []
        if q0 < 512:
            out.append((q0, 512))
            out.append((512, 1024))
        else:
            out.append((q0, 1024))
        return out

    with tile.TileContext(nc) as tc:
        with (
            tc.tile_pool(name="const", bufs=1) as cpool,
            tc.tile_pool(name="big", bufs=1) as bigpool,
        ):
            hm2_sb = cpool.tile([1, 2, P], BF16)

            v = bigpool.tile([P, NB, H, DK + 1], BF16)
            aT = bigpool.tile([P, CB, SLOTS * P], BF16)
            vg = v[:]
            nc.gpsimd.dma_start(hm2_sb[:], hm2[:])
            nc.vector.memset(vg[:, :, :, DK : DK + 1], 1.0)

            # ---- outer loop pools (survive into the O phase) ----
            with (
                tc.tile_pool(name="expS", bufs=18) as spool,
                tc.tile_pool(name="osb", bufs=2) as osbp,
                tc.tile_pool(name="dnp", bufs=1) as dnp,
                tc.tile_pool(name="rsb", bufs=1) as rsbp,
                tc.tile_pool(name="ps_av", bufs=1, space="PSUM") as ps_av,
                tc.tile_pool(name="ps_r", bufs=1, space="PSUM") as ps_r,
                tc.tile_pool(name="xbig", bufs=1) as xbig,
            ):
                masks_sb = xbig.tile([P, SLOTS, 2, P], BF16)
                bq_sb = xbig.tile([P, CB], F32)
                bk_sb = xbig.tile([P, CB], F32)
                xT_sb = xbig.tile([P, CB, T], BF16)
                xTq_sb = xbig.tile([P, CB, SLOTS * P], BF16)

                # xT first: phase V is the first PE consumer
                nc.gpsimd.dma_start(bq_sb[:], bq[:])
                nc.gpsimd.dma_start(bk_sb[:], bk[:])

                # ---- phase V: value projection, all heads ----
                with (
                    tc.tile_pool(name="wv", bufs=1) as wvpool,
                    tc.tile_pool(name="psv", bufs=1, space="PSUM") as psv,
                ):
                    wv_sb = wvpool.tile([P, CB, C], BF16)
                    nc.gpsimd.dma_start(
                        wv_sb[:], wvT.rearrange("(ko p) n -> p ko n", p=P)
                    )
                    xTr = xT.rearrange("(ko p) t -> p ko t", p=P)
                    nc.gpsimd.dma_start(xT_sb[:, :, 0:1024], xTr[:, :, 0:1024])
                    nc.gpsimd.dma_start(xT_sb[:, :, 1024:T], xTr[:, :, 1024:T])
                    nc.gpsimd.dma_start(
                        xTq_sb[:], xTq.rearrange("(ko p) t -> p ko t", p=P)
                    )
                    nc.gpsimd.dma_start(
                        masks_sb[:], masks[:].rearrange("s t p q -> p s t q")
                    )
                    for tbg in range(NB // 2):
                        acc = [
                            psv.tile([P, 512], F32, tag=f"vacc{i}", name=f"vacc{i}")
                            for i in range(4)
                        ]
                        for kb in range(CB):
                            for ti in range(2):
                                tb = tbg * 2 + ti
                                for dch in range(2):
                                    nc.tensor.matmul(
                                        acc[ti * 2 + dch][:],
                                        xT_sb[:, kb, ts(tb, P)],
                                        wv_sb[:, kb, ts(dch, 512)],
                                        start=(kb == 0),
                                        stop=(kb == CB - 1),
                                    )
                        for ti in range(2):
                            tb = tbg * 2 + ti
                            for dch in range(2):
                                nc.scalar.copy(
                                    vg[:, tb, dch * 8 : (dch + 1) * 8, 0:DK],
                                    acc[ti * 2 + dch][:].rearrange(
                                        "p (h e) -> p h e", e=DK
                                    ),
                                )

                # ---- main pipelined loop over head pairs ----
                inner2 = [
                    tc.tile_pool(name="wqk", bufs=2),
                    tc.tile_pool(name="kq", bufs=2),
                    tc.tile_pool(name="ps_sc", bufs=2, space="PSUM"),
                    tc.tile_pool(name="ps_pj", bufs=1, space="PSUM"),
                ]
                wqk = inner2[0].__enter__()
                kq = inner2[1].__enter__()
                ps_sc = inner2[2].__enter__()
                ps_pj = inner2[3].__enter__()
                kT = {}
                qT = {}
                expS = {}

                def dma_weights(c):
                    wq_t = wqk.tile([P, CB, P], BF16, tag="wq", name=f"wq{c}")
                    wk_t = wqk.tile([P, CB, P], BF16, tag="wk", name=f"wk{c}")
                    nc.gpsimd.dma_start(
                        wq_t[:],
                        wqT.rearrange("(ko p) n -> p ko n", p=P)[
                            :, :, ts(c, P)
                        ],
                    )
                    nc.gpsimd.dma_start(
                        wk_t[:],
                        wkT.rearrange("(ko p) n -> p ko n", p=P)[
                            :, :, ts(c, P)
                        ],
                    )
                    return wq_t, wk_t

                wtiles = {0: dma_weights(0)}

                def qk_units(c):
                    """Generator of PE units for pair-c Q/K projection."""
                    wq_t, wk_t = wtiles[c]
                    qT[c] = kq.tile([P, SLOTS * P], BF16, tag="qT", name=f"qT{c}")
                    kT[c] = kq.tile([P, T], BF16, tag="kT", name=f"kT{c}")
                    for dst, w_t, src_t, nnch, bias in (
                        (qT[c], wq_t, xTq_sb, 2, bq_sb),
                        (kT[c], wk_t, xT_sb, 4, bk_sb),
                    ):
                        for nch in range(nnch):
                            acc = ps_pj.tile([P, 512], F32, tag="pj")
                            for kb in range(CB):
                                def mm(kb=kb, acc=acc, w_t=w_t, src_t=src_t, nch=nch):
                                    nc.tensor.matmul(
                                        acc[:],
                                        w_t[:, kb, :],
                                        src_t[:, kb, ts(nch, 512)],
                                        start=(kb == 0),
                                        stop=(kb == CB - 1),
                                    )
                                yield mm
                            def drain(acc=acc, dst=dst, nch=nch, bias=bias):
                                nc.vector.tensor_scalar_add(
                                    dst[:, ts(nch, 512)], acc[:], bias[:, c : c + 1]
                                )
                            yield drain

                def sc_groups(c):
                    """List of per-jb closures: scores (row-tiled) + exp + mask."""
                    expS[c] = [
                        spool.tile([P, 2, SLOTS * P], BF16, tag="expS",
                                   name=f"expS{c}_{j}")
                        for j in range(NB)
                    ]
                    groups = []
                    for jb in ILV:
                        def grp(jb=jb, c=c):
                            sm = jb // 2
                            for ci, (qa, qb) in enumerate(qchunks(jb)):
                                w = qb - qa
                                pss = ps_sc.tile([P, 2, 512], F32, tag="sc")
                                for h in range(2):
                                    nc.tensor.matmul(
                                        pss[:, h, 0:w],
                                        kT[c][h * DK : (h + 1) * DK, ts(jb, P)],
                                        qT[c][h * DK : (h + 1) * DK, qa:qb],
                                        start=True,
                                        stop=True,
                                    )
                                nc.scalar.activation(
                                    expS[c][jb][:, :, qa:qb],
                                    pss[:, :, 0:w],
                                    EXP,
                                    scale=float(SCALE),
                                )
                                if ci == 0:
                                    for h in range(2):
                                        nc.vector.tensor_mul(
                                            expS[c][jb][:, h, ts(sm, P)],
                                            expS[c][jb][:, h, ts(sm, P)],
                                            masks_sb[:, sm, jb % 2, :],
                                        )
                        groups.append(grp)
                    return groups

                def attnv_units(c):
                    """Generator of PE units for pair-c attnv + normalization."""
                    for half in range(2):
                        qlo, qhi = half * 512, half * 512 + 512
                        pso = ps_av.tile([DK + 1, 2, 512], F32, tag="av")
                        jbs = list(range(8)) if half == 0 else list(ILV)
                        for idx, jb in enumerate(jbs):
                            q0 = P * (jb // 2)
                            qa = max(q0, qlo)
                            for h in range(2):
                                def mm(jb=jb, h=h, qa=qa, pso=pso, idx=idx,
                                       c=c, qlo=qlo, qhi=qhi, last=(idx == len(jbs) - 1)):
                                    nc.tensor.matmul(
                                        pso[:, h, qa - qlo : qhi - qlo],
                                        v[:, jb, 2 * c + h, :],
                                        expS[c][jb][:, h, qa:qhi],
                                        start=(idx == 0),
                                        stop=last,
                                    )
                                yield mm
                        def drain(pso=pso, half=half, qlo=qlo, c=c):
                            # stage O^T per-head to partitions 0:64/64:128 and
                            # the denom row to sbuf, releasing the psum banks
                            osb = osbp.tile([P, 512], BF16, tag="osb")
                            den = dnp.tile([1, 2, 512], BF16, tag="den")
                            nc.scalar.copy(den[:], pso[DK : DK + 1, :, :])
                            nc.vector.tensor_copy(osb[0:DK, :], pso[0:DK, 0, :])
                            nc.scalar.copy(osb[DK:P, :], pso[0:DK, 1, :])
                            # broadcast denoms (rows 0:64 <- head0, 64:128 <- head1)
                            psr = ps_r.tile([P, 512], F32, tag="r")
                            for h in range(2):
                                nc.tensor.matmul(
                                    psr[:],
                                    hm2_sb[0:1, h, :],
                                    den[0:1, h, :],
                                    start=(h == 0),
                                    stop=(h == 1),
                                )
                            r_sb = rsbp.tile([P, 512], F32, tag="rsb")
                            nc.vector.reciprocal_approx_fast(r_sb[:], psr[:])
                            for h in range(2):
                                nc.vector.tensor_mul(
                                    aT[h * DK : (h + 1) * DK, c, qlo : qlo + 512],
                                    osb[h * DK : (h + 1) * DK, :],
                                    r_sb[h * DK : (h + 1) * DK, :],
                                )
                        yield drain

                # ---- pipeline schedule ----
                POPS_AV = [7, 7, 6, 6, 5, 5, 5, 5, 2, 2, 2, 2, 1, 1, 0, 0]
                POPS_QK = [4, 4, 4, 4, 4, 4, 3, 3, 3, 3, 3, 3, 2, 2, 2, 2]

                def tick(sc_c, qk_c, av_c):
                    """One pipeline tick: interleave pair-sc_c scores with
                    pair-qk_c projections and pair-av_c attnv on the PE queue.
                    attnv pops are front-loaded so expS slot releases always
                    precede the exp writes that reuse them (no sem cycle)."""
                    sgs = sc_groups(sc_c)
                    qg = iter(qk_units(qk_c)) if qk_c is not None else iter(())
                    ag = iter(attnv_units(av_c)) if av_c is not None else iter(())
                    for g in range(NB):
                        sgs[g]()
                        _pop(qg, POPS_QK[g])
                        _pop(ag, POPS_AV[g])
                    _pop(qg, 99)
                    _pop(ag, 99)

                # tick 1: QK(0) dense; prefetch w(1)
                for u in qk_units(0):
                    u()
                wtiles[1] = dma_weights(1)
                # tick 2: QK(1) x sc(0)
                tick(0, 1, None)
                wtiles[2] = dma_weights(2)
                # ticks 3..8: QK(c+2) x sc(c+1) x attnv(c)
                for c in range(6):
                    tick(c + 1, c + 2, c)
                    if c + 3 < CB:
                        wtiles[c + 3] = dma_weights(c + 3)
                # tick 9: sc(7) x attnv(6)
                tick(7, None, 6)
                for pool in reversed(inner2):
                    pool.__exit__(None, None, None)

                # ---- output projection, overlapped with attnv(7) ----
                with (
                    tc.tile_pool(name="out", bufs=1) as opool,
                    tc.tile_pool(name="yt", bufs=2) as ytp,
                    tc.tile_pool(name="ps_y", bufs=2, space="PSUM") as ps_y,
                ):
                    woT_sb = opool.tile([P, CB, C], BF16)
                    nc.gpsimd.dma_start(
                        woT_sb[:], woT.rearrange("(ko p) n -> p ko n", p=P)
                    )

                    def o_units():
                        for tb in range(SLOTS):
                            y_t = ytp.tile([P, C], F32, tag="yt", name=f"y{tb}")
                            for nch in range(2):
                                psy = ps_y.tile([P, 512], F32, tag="ps_y")
                                for cbk in range(CB):
                                    def mm(psy=psy, cbk=cbk, tb=tb, nch=nch):
                                        nc.tensor.matmul(
                                            psy[:],
                                            aT[:, cbk, ts(tb, P)],
                                            woT_sb[:, cbk, ts(nch, 512)],
                                            start=(cbk == 0),
                                            stop=(cbk == CB - 1),
                                        )
                                    yield mm
                                def drain(psy=psy, y_t=y_t, nch=nch):
                                    nc.vector.tensor_copy(y_t[:, ts(nch, 512)], psy[:])
                                yield drain
                            def dma(y_t=y_t, tb=tb):
                                nc.gpsimd.dma_start(
                                    y.rearrange("(tb p) c -> p tb c", p=P)[:, tb, :],
                                    y_t[:],
                                )
                            yield dma

                    ag7 = iter(attnv_units(7))
                    og = iter(o_units())
                    _pop(ag7, 17)   # half0 MMs + its norm drain first
                    for _ in range(33):
                        _pop(ag7, 1)
                        _pop(og, 2)
                    _pop(og, 999)

    nc.compile()
    return nc


def _host_inputs(x, mask, Wq, bq_v, Wk, bk_v, Wv, bv_v, Wo, bo_v):
    """Per-core input maps + the host-side output bias correction."""
    f32 = np.float32
    bf16 = ml_dtypes.bfloat16
    wqT = np.ascontiguousarray(np.asarray(Wq, f32).T).astype(bf16)
    wkT = np.ascontiguousarray(np.asarray(Wk, f32).T).astype(bf16)
    wvT = np.ascontiguousarray(np.asarray(Wv, f32).T).astype(bf16)
    woT = np.ascontiguousarray(np.asarray(Wo, f32).T).astype(bf16)
    bq_p = np.ascontiguousarray(np.asarray(bq_v, f32).reshape(C // P, P).T)
    bk_p = np.ascontiguousarray(np.asarray(bk_v, f32).reshape(C // P, P).T)
    # exact v/o bias fold: softmax rows sum to 1, so v+bv adds bv to attn out
    bo_eff = (np.asarray(bo_v, f32) + np.asarray(bv_v, f32) @ np.asarray(Wo, f32).T)
    # hm2: rank-2 broadcast matrix for per-head reciprocal rows
    hm2_np = np.zeros((1, 2, P), f32)
    hm2_np[0, 0, 0:DK] = 1.0
    hm2_np[0, 1, DK:P] = 1.0
    hm2_np = hm2_np.astype(bf16)

    # per-half causal boundary masks for the last two key blocks of each slot
    mask_half = []
    tri = np.tril(np.ones((P, P), f32)).T  # [j, i] = 1 where j <= i
    for half in range(2):
        m = np.zeros((SLOTS, 2, P, P), f32)
        for s in range(SLOTS):
            g = QBLKS[half][s]
            for idx, jb in enumerate((2 * s, 2 * s + 1)):
                if jb < g:
                    m[s, idx] = 1.0
                elif jb == g:
                    m[s, idx] = tri
        mask_half.append(m.astype(bf16))

    xn = np.asarray(x, f32)
    in_maps = []
    for core in range(8):
        b, half = divmod(core, 2)
        xT = np.ascontiguousarray(xn[b].T).astype(bf16)
        qtok = np.concatenate([np.arange(g * P, (g + 1) * P) for g in QBLKS[half]])
        xTq = np.ascontiguousarray(xn[b][qtok].T).astype(bf16)
        in_maps.append(
            {
                "xT": xT,
                "xTq": xTq,
                "wqT": wqT,
                "wkT": wkT,
                "wvT": wvT,
                "woT": woT,
                "bq": bq_p,
                "bk": bk_p,
                "masks": mask_half[half],
                "hm2": hm2_np,
            }
        )
    return in_maps, bo_eff


def _run(inputs, trace=False):
    if "nc" not in _cache:
        _cache["nc"] = _build()
    nc = _cache["nc"]
    in_maps, bo_eff = _host_inputs(
        inputs["x"], inputs["mask"],
        inputs["Wq"], inputs["bq"], inputs["Wk"], inputs["bk"],
        inputs["Wv"], inputs["bv"], inputs["Wo"], inputs["bo"],
    )
    res = run_bass_kernel_spmd(nc, in_maps, list(range(8)), trace=trace)
    out = np.empty((B, T, C), np.float32)
    for core in range(8):
        b, half = divmod(core, 2)
        yc = res.results[core]["y"]
        for s, g in enumerate(QBLKS[half]):
            out[b, g * P : (g + 1) * P] = yc[s * P : (s + 1) * P]
    out += bo_eff
    return out, res


def kernel(**inputs):
    out, _ = _run(inputs, trace=False)
    return out


# revision 4
# speedup vs baseline: 1.0132x; 1.0132x over previous
"""Multi-head causal attention (B=4, T=2048, C=1024, H=16) on 8 trn2 cores.

Sharding: data-parallel over batch (4) x sequence-parallel over causal query
blocks (2), zig-zag balanced: core = 2*b + half; half 0 gets query blocks
[0,2,4,6,9,11,13,15], half 1 gets [1,3,5,7,8,10,12,14]. Slot s processes
key blocks 0..2s+1; causal boundary via per-core input masks.

Kernel structure (v2): software-pipelined over head pairs c=0..7.
  tick: [Q/K proj pair c+1] x [scores+exp+mask pair c] x [attnv pair c-1]
Scores are row-tiled (K=64, two heads in PE row strips 0/64 concurrently).
attnv is v-stationary (v tile [128,65] incl ones column -> softmax denom in
psum row 64), streaming wide query chunks; output lands as O^T = aT layout
(no transpose phase). Normalization: reciprocal of denom row + rank-2
broadcast matmul (hm2) + partition-shifted DVE multiplies into aT.
"""

import numpy as np
import ml_dtypes

import concourse.bass as bass
import concourse.mybir as mybir
import concourse.tile as tile
from concourse import bacc
from concourse.bass import ts
from concourse.bass_utils import run_bass_kernel_spmd

B, T, C, H, DK = 4, 2048, 1024, 16, 64
P = 128
NB = T // P          # 16 key blocks
SLOTS = 8            # query blocks per core
CB = C // P          # 8 channel blocks = head pairs
SCALE = 1.0 / np.sqrt(DK)
BF16 = mybir.dt.bfloat16
F32 = mybir.dt.float32
EXP = mybir.ActivationFunctionType.Exp

QBLKS = [
    [0, 2, 4, 6, 9, 11, 13, 15],
    [1, 3, 5, 7, 8, 10, 12, 14],
]

# interleaved jb order: evens ACT load per group and matches attnv's
# half-1 read order so expS slot releases track allocations
ILV = [0, 8, 1, 9, 2, 10, 3, 11, 4, 12, 5, 13, 6, 14, 7, 15]

_cache = {}


def _pop(gen, n):
    for _ in range(n):
        try:
            next(gen)()
        except StopIteration:
            return


def _build():
    nc = bacc.Bacc("TRN2", target_bir_lowering=False, debug=False)

    xT = nc.dram_tensor("xT", [C, T], BF16, kind="ExternalInput").ap()
    xTq = nc.dram_tensor("xTq", [C, SLOTS * P], BF16, kind="ExternalInput").ap()
    wqT = nc.dram_tensor("wqT", [C, C], BF16, kind="ExternalInput").ap()
    wkT = nc.dram_tensor("wkT", [C, C], BF16, kind="ExternalInput").ap()
    wvT = nc.dram_tensor("wvT", [C, C], BF16, kind="ExternalInput").ap()
    woT = nc.dram_tensor("woT", [C, C], BF16, kind="ExternalInput").ap()
    bq = nc.dram_tensor("bq", [P, CB], F32, kind="ExternalInput").ap()
    bk = nc.dram_tensor("bk", [P, CB], F32, kind="ExternalInput").ap()
    masks = nc.dram_tensor("masks", [SLOTS, 2, P, P], BF16, kind="ExternalInput").ap()
    hm2 = nc.dram_tensor("hm2", [1, 2, P], BF16, kind="ExternalInput").ap()
    y = nc.dram_tensor("y", [SLOTS * P, C], F32, kind="ExternalOutput").ap()

    def qchunks(jb):
        """score q-chunks for key block jb: [q0,512) and/or [512,1024)."""
        q0 = P * (jb // 2)
        out = []
        if q0 < 512:
            out.append((q0, 512))
            out.append((512, 1024))
        else:
            out.append((q0, 1024))
        return out

    with tile.TileContext(nc) as tc:
        with (
            tc.tile_pool(name="const", bufs=1) as cpool,
            tc.tile_pool(name="big", bufs=1) as bigpool,
        ):
            hm2_sb = cpool.tile([1, 2, P], BF16)

            v = bigpool.tile([P, NB, H, DK + 1], BF16)
            aT = {
                c: bigpool.tile([P, SLOTS * P], BF16, tag=f"aT{c}",
                                name=f"aT{c}")
                for c in range(CB)
            }
            vg = v[:]
            nc.gpsimd.dma_start(hm2_sb[:], hm2[:])
            nc.vector.memset(vg[:, :, :, DK : DK + 1], 1.0)

            # ---- outer loop pools (survive into the O phase) ----
            with (
                tc.tile_pool(name="expS", bufs=18) as spool,
                tc.tile_pool(name="osb", bufs=2) as osbp,
                tc.tile_pool(name="dnp", bufs=1) as dnp,
                tc.tile_pool(name="rsb", bufs=1) as rsbp,
                tc.tile_pool(name="ps_av", bufs=1, space="PSUM") as ps_av,
                tc.tile_pool(name="ps_r", bufs=1, space="PSUM") as ps_r,
                tc.tile_pool(name="xbig", bufs=1) as xbig,
            ):
                masks_sb = xbig.tile([P, SLOTS, 2, P], BF16)
                bq_sb = xbig.tile([P, CB], F32)
                bk_sb = xbig.tile([P, CB], F32)
                xT_sb = xbig.tile([P, CB, T], BF16)
                xTq_sb = xbig.tile([P, CB, SLOTS * P], BF16)

                # xT first: phase V is the first PE consumer
                nc.gpsimd.dma_start(bq_sb[:], bq[:])
                nc.gpsimd.dma_start(bk_sb[:], bk[:])

                # ---- phase V: value projection, all heads ----
                with (
                    tc.tile_pool(name="wv", bufs=1) as wvpool,
                    tc.tile_pool(name="psv", bufs=1, space="PSUM") as psv,
                ):
                    wv_sb = wvpool.tile([P, CB, C], BF16)
                    wvr = wvT.rearrange("(ko p) n -> p ko n", p=P)
                    xTr = xT.rearrange("(ko p) t -> p ko t", p=P)
                    nc.gpsimd.dma_start(wv_sb[:, 0:4, :], wvr[:, 0:4, :])
                    nc.gpsimd.dma_start(xT_sb[:, 0:4, 0:512], xTr[:, 0:4, 0:512])
                    nc.gpsimd.dma_start(wv_sb[:, 4:8, :], wvr[:, 4:8, :])
                    nc.gpsimd.dma_start(xT_sb[:, 4:8, 0:512], xTr[:, 4:8, 0:512])
                    nc.gpsimd.dma_start(xT_sb[:, :, 512:1024], xTr[:, :, 512:1024])
                    nc.gpsimd.dma_start(xT_sb[:, :, 1024:T], xTr[:, :, 1024:T])
                    nc.gpsimd.dma_start(
                        xTq_sb[:], xTq.rearrange("(ko p) t -> p ko t", p=P)
                    )
                    nc.gpsimd.dma_start(
                        masks_sb[:], masks[:].rearrange("s t p q -> p s t q")
                    )
                    for tbg in range(NB // 2):
                        acc = [
                            psv.tile([P, 512], F32, tag=f"vacc{i}", name=f"vacc{i}")
                            for i in range(4)
                        ]
                        for kb in range(CB):
                            for ti in range(2):
                                tb = tbg * 2 + ti
                                for dch in range(2):
                                    nc.tensor.matmul(
                                        acc[ti * 2 + dch][:],
                                        xT_sb[:, kb, ts(tb, P)],
                                        wv_sb[:, kb, ts(dch, 512)],
                                        start=(kb == 0),
                                        stop=(kb == CB - 1),
                                    )
                        for ti in range(2):
                            tb = tbg * 2 + ti
                            for dch in range(2):
                                eng = nc.scalar.copy if dch == 0 else (
                                    nc.vector.tensor_copy)
                                eng(
                                    vg[:, tb, dch * 8 : (dch + 1) * 8, 0:DK],
                                    acc[ti * 2 + dch][:].rearrange(
                                        "p (h e) -> p h e", e=DK
                                    ),
                                )

                # ---- main pipelined loop over head pairs ----
                inner2 = [
                    tc.tile_pool(name="wqk", bufs=2),
                    tc.tile_pool(name="kq", bufs=2),
                    tc.tile_pool(name="ps_sc", bufs=2, space="PSUM"),
                    tc.tile_pool(name="ps_pj", bufs=1, space="PSUM"),
                ]
                wqk = inner2[0].__enter__()
                kq = inner2[1].__enter__()
                ps_sc = inner2[2].__enter__()
                ps_pj = inner2[3].__enter__()
                kT = {}
                qT = {}
                expS = {}

                def dma_weights(c):
                    wq_t = wqk.tile([P, CB, P], BF16, tag="wq", name=f"wq{c}")
                    wk_t = wqk.tile([P, CB, P], BF16, tag="wk", name=f"wk{c}")
                    nc.gpsimd.dma_start(
                        wq_t[:],
                        wqT.rearrange("(ko p) n -> p ko n", p=P)[
                            :, :, ts(c, P)
                        ],
                    )
                    nc.gpsimd.dma_start(
                        wk_t[:],
                        wkT.rearrange("(ko p) n -> p ko n", p=P)[
                            :, :, ts(c, P)
                        ],
                    )
                    return wq_t, wk_t

                wtiles = {0: dma_weights(0)}

                def qk_units(c):
                    """Generator of PE units for pair-c Q/K projection."""
                    wq_t, wk_t = wtiles[c]
                    qT[c] = kq.tile([P, SLOTS * P], BF16, tag="qT", name=f"qT{c}")
                    kT[c] = kq.tile([P, T], BF16, tag="kT", name=f"kT{c}")
                    for dst, w_t, src_t, nnch, bias in (
                        (qT[c], wq_t, xTq_sb, 2, bq_sb),
                        (kT[c], wk_t, xT_sb, 4, bk_sb),
                    ):
                        for nch in range(nnch):
                            acc = ps_pj.tile([P, 512], F32, tag="pj")
                            for kb in range(CB):
                                def mm(kb=kb, acc=acc, w_t=w_t, src_t=src_t, nch=nch):
                                    nc.tensor.matmul(
                                        acc[:],
                                        w_t[:, kb, :],
                                        src_t[:, kb, ts(nch, 512)],
                                        start=(kb == 0),
                                        stop=(kb == CB - 1),
                                    )
                                yield mm
                            def drain(acc=acc, dst=dst, nch=nch, bias=bias):
                                nc.vector.tensor_scalar_add(
                                    dst[:, ts(nch, 512)], acc[:], bias[:, c : c + 1]
                                )
                            yield drain

                def sc_groups(c):
                    """List of per-jb closures: scores (row-tiled) + exp + mask."""
                    expS[c] = [
                        spool.tile([P, 2, SLOTS * P], BF16, tag="expS",
                                   name=f"expS{c}_{j}")
                        for j in range(NB)
                    ]
                    groups = []
                    for jb in ILV:
                        def grp(jb=jb, c=c):
                            sm = jb // 2
                            for ci, (qa, qb) in enumerate(qchunks(jb)):
                                w = qb - qa
                                pss = ps_sc.tile([P, 2, 512], F32, tag="sc")
                                for h in range(2):
                                    nc.tensor.matmul(
                                        pss[:, h, 0:w],
                                        kT[c][h * DK : (h + 1) * DK, ts(jb, P)],
                                        qT[c][h * DK : (h + 1) * DK, qa:qb],
                                        start=True,
                                        stop=True,
                                    )
                                nc.scalar.activation(
                                    expS[c][jb][:, :, qa:qb],
                                    pss[:, :, 0:w],
                                    EXP,
                                    scale=float(SCALE),
                                )
                                if ci == 0:
                                    for h in range(2):
                                        nc.vector.tensor_mul(
                                            expS[c][jb][:, h, ts(sm, P)],
                                            expS[c][jb][:, h, ts(sm, P)],
                                            masks_sb[:, sm, jb % 2, :],
                                        )
                        groups.append(grp)
                    return groups

                def attnv_units(c):
                    """Generator of PE units for pair-c attnv + normalization."""
                    for half in range(2):
                        qlo, qhi = half * 512, half * 512 + 512
                        pso = ps_av.tile([DK + 1, 2, 512], F32, tag="av")
                        jbs = list(range(8)) if half == 0 else list(ILV)
                        for idx, jb in enumerate(jbs):
                            q0 = P * (jb // 2)
                            qa = max(q0, qlo)
                            for h in range(2):
                                def mm(jb=jb, h=h, qa=qa, pso=pso, idx=idx,
                                       c=c, qlo=qlo, qhi=qhi, last=(idx == len(jbs) - 1)):
                                    nc.tensor.matmul(
                                        pso[:, h, qa - qlo : qhi - qlo],
                                        v[:, jb, 2 * c + h, :],
                                        expS[c][jb][:, h, qa:qhi],
                                        start=(idx == 0),
                                        stop=last,
                                    )
                                yield mm
                        def drain(pso=pso, half=half, qlo=qlo, c=c):
                            # stage O^T per-head to partitions 0:64/64:128 and
                            # the denom row to sbuf, releasing the psum banks
                            osb = osbp.tile([P, 512], BF16, tag="osb")
                            den = dnp.tile([1, 2, 512], BF16, tag="den")
                            nc.scalar.copy(den[:], pso[DK : DK + 1, :, :])
                            nc.vector.tensor_copy(osb[0:DK, :], pso[0:DK, 0, :])
                            nc.scalar.copy(osb[DK:P, :], pso[0:DK, 1, :])
                            # broadcast denoms (rows 0:64 <- head0, 64:128 <- head1)
                            psr = ps_r.tile([P, 512], F32, tag="r")
                            for h in range(2):
                                nc.tensor.matmul(
                                    psr[:],
                                    hm2_sb[0:1, h, :],
                                    den[0:1, h, :],
                                    start=(h == 0),
                                    stop=(h == 1),
                                )
                            r_sb = rsbp.tile([P, 512], F32, tag="rsb")
                            nc.vector.reciprocal_approx_fast(r_sb[:], psr[:])
                            for h in range(2):
                                nc.vector.tensor_mul(
                                    aT[c][h * DK : (h + 1) * DK, qlo : qlo + 512],
                                    osb[h * DK : (h + 1) * DK, :],
                                    r_sb[h * DK : (h + 1) * DK, :],
                                )
                        yield drain

                # ---- pipeline schedule ----
                POPS_AV = [7, 7, 6, 6, 5, 5, 5, 5, 2, 2, 2, 2, 1, 1, 0, 0]
                POPS_QK = [4, 4, 4, 4, 4, 4, 3, 3, 3, 3, 3, 3, 2, 2, 2, 2]

                def tick(sc_c, qk_c, av_c):
                    """One pipeline tick: interleave pair-sc_c scores with
                    pair-qk_c projections and pair-av_c attnv on the PE queue.
                    attnv pops are front-loaded so expS slot releases always
                    precede the exp writes that reuse them (no sem cycle)."""
                    sgs = sc_groups(sc_c)
                    qg = iter(qk_units(qk_c)) if qk_c is not None else iter(())
                    ag = iter(attnv_units(av_c)) if av_c is not None else iter(())
                    for g in range(NB):
                        sgs[g]()
                        _pop(qg, POPS_QK[g])
                        _pop(ag, POPS_AV[g])
                    _pop(qg, 99)
                    _pop(ag, 99)

                # tick 1: QK(0) dense; prefetch w(1)
                for u in qk_units(0):
                    u()
                wtiles[1] = dma_weights(1)
                # tick 2: QK(1) x sc(0)
                tick(0, 1, None)
                wtiles[2] = dma_weights(2)
                # ticks 3..8: QK(c+2) x sc(c+1) x attnv(c)
                for c in range(6):
                    tick(c + 1, c + 2, c)
                    if c + 3 < CB:
                        wtiles[c + 3] = dma_weights(c + 3)
                # tick 9: sc(7) x attnv(6)
                tick(7, None, 6)
                for pool in reversed(inner2):
                    pool.__exit__(None, None, None)

                # ---- output projection, overlapped with attnv(7) ----
                with (
                    tc.tile_pool(name="out", bufs=1) as opool,
                    tc.tile_pool(name="yt", bufs=2) as ytp,
                    tc.tile_pool(name="ps_y", bufs=2, space="PSUM") as ps_y,
                ):
                    woT_sb = opool.tile([P, CB, C], BF16)
                    nc.gpsimd.dma_start(
                        woT_sb[:], woT.rearrange("(ko p) n -> p ko n", p=P)
                    )

                    def o_units():
                        for tb in range(SLOTS):
                            y_t = ytp.tile([P, C], F32, tag="yt", name=f"y{tb}")
                            for nch in range(2):
                                psy = ps_y.tile([P, 512], F32, tag="ps_y")
                                for cbk in range(CB):
                                    def mm(psy=psy, cbk=cbk, tb=tb, nch=nch):
                                        nc.tensor.matmul(
                                            psy[:],
                                            aT[cbk][:, ts(tb, P)],
                                            woT_sb[:, cbk, ts(nch, 512)],
                                            start=(cbk == 0),
                                            stop=(cbk == CB - 1),
                                        )
                                    yield mm
                                def drain(psy=psy, y_t=y_t, nch=nch):
                                    nc.vector.tensor_copy(y_t[:, ts(nch, 512)], psy[:])
                                yield drain
                            def dma(y_t=y_t, tb=tb):
                                nc.gpsimd.dma_start(
                                    y.rearrange("(tb p) c -> p tb c", p=P)[:, tb, :],
                                    y_t[:],
                                )
                            yield dma

                    ag7 = iter(attnv_units(7))
                    og = iter(o_units())
                    _pop(ag7, 17)   # half0 MMs + its norm drain first
                    for _ in range(33):
                        _pop(ag7, 1)
                        _pop(og, 2)
                    _pop(og, 999)

    nc.compile()
    return nc


def _host_inputs(x, mask, Wq, bq_v, Wk, bk_v, Wv, bv_v, Wo, bo_v):
    """Per-core input maps + the host-side output bias correction."""
    f32 = np.float32
    bf16 = ml_dtypes.bfloat16
    wqT = np.ascontiguousarray(np.asarray(Wq, f32).T).astype(bf16)
    wkT = np.ascontiguousarray(np.asarray(Wk, f32).T).astype(bf16)
    wvT = np.ascontiguousarray(np.asarray(Wv, f32).T).astype(bf16)
    woT = np.ascontiguousarray(np.asarray(Wo, f32).T).astype(bf16)
    bq_p = np.ascontiguousarray(np.asarray(bq_v, f32).reshape(C // P, P).T)
    bk_p = np.ascontiguousarray(np.asarray(bk_v, f32).reshape(C // P, P).T)
    # exact v/o bias fold: softmax rows sum to 1, so v+bv adds bv to attn out
    bo_eff = (np.asarray(bo_v, f32) + np.asarray(bv_v, f32) @ np.asarray(Wo, f32).T)
    # hm2: rank-2 broadcast matrix for per-head reciprocal rows
    hm2_np = np.zeros((1, 2, P), f32)
    hm2_np[0, 0, 0:DK] = 1.0
    hm2_np[0, 1, DK:P] = 1.0
    hm2_np = hm2_np.astype(bf16)

    # per-half causal boundary masks for the last two key blocks of each slot
    mask_half = []
    tri = np.tril(np.ones((P, P), f32)).T  # [j, i] = 1 where j <= i
    for half in range(2):
        m = np.zeros((SLOTS, 2, P, P), f32)
        for s in range(SLOTS):
            g = QBLKS[half][s]
            for idx, jb in enumerate((2 * s, 2 * s + 1)):
                if jb < g:
                    m[s, idx] = 1.0
                elif jb == g:
                    m[s, idx] = tri
        mask_half.append(m.astype(bf16))

    xn = np.asarray(x, f32)
    in_maps = []
    for core in range(8):
        b, half = divmod(core, 2)
        xT = np.ascontiguousarray(xn[b].T).astype(bf16)
        qtok = np.concatenate([np.arange(g * P, (g + 1) * P) for g in QBLKS[half]])
        xTq = np.ascontiguousarray(xn[b][qtok].T).astype(bf16)
        in_maps.append(
            {
                "xT": xT,
                "xTq": xTq,
                "wqT": wqT,
                "wkT": wkT,
                "wvT": wvT,
                "woT": woT,
                "bq": bq_p,
                "bk": bk_p,
                "masks": mask_half[half],
                "hm2": hm2_np,
            }
        )
    return in_maps, bo_eff


def _run(inputs, trace=False):
    if "nc" not in _cache:
        _cache["nc"] = _build()
    nc = _cache["nc"]
    in_maps, bo_eff = _host_inputs(
        inputs["x"], inputs["mask"],
        inputs["Wq"], inputs["bq"], inputs["Wk"], inputs["bk"],
        inputs["Wv"], inputs["bv"], inputs["Wo"], inputs["bo"],
    )
    res = run_bass_kernel_spmd(nc, in_maps, list(range(8)), trace=trace)
    out = np.empty((B, T, C), np.float32)
    for core in range(8):
        b, half = divmod(core, 2)
        yc = res.results[core]["y"]
        for s, g in enumerate(QBLKS[half]):
            out[b, g * P : (g + 1) * P] = yc[s * P : (s + 1) * P]
    out += bo_eff
    return out, res


def kernel(**inputs):
    out, _ = _run(inputs, trace=False)
    return out


# revision 5
# speedup vs baseline: 1.0183x; 1.0050x over previous
"""Multi-head causal attention (B=4, T=2048, C=1024, H=16) on 8 trn2 cores.

Sharding: data-parallel over batch (4) x sequence-parallel over causal query
blocks (2), zig-zag balanced: core = 2*b + half; half 0 gets query blocks
[0,2,4,6,9,11,13,15], half 1 gets [1,3,5,7,8,10,12,14]. Slot s processes
key blocks 0..2s+1; causal boundary via per-core input masks.

Kernel structure (v2): software-pipelined over head pairs c=0..7.
  tick: [Q/K proj pair c+1] x [scores+exp+mask pair c] x [attnv pair c-1]
Scores are row-tiled (K=64, two heads in PE row strips 0/64 concurrently).
attnv is v-stationary (v tile [128,65] incl ones column -> softmax denom in
psum row 64), streaming wide query chunks; output lands as O^T = aT layout
(no transpose phase). Normalization: reciprocal of denom row + rank-2
broadcast matmul (hm2) + partition-shifted DVE multiplies into aT.
"""

import numpy as np
import ml_dtypes

import concourse.bass as bass
import concourse.mybir as mybir
import concourse.tile as tile
from concourse import bacc
from concourse.bass import ts
from concourse.bass_utils import run_bass_kernel_spmd

B, T, C, H, DK = 4, 2048, 1024, 16, 64
P = 128
NB = T // P          # 16 key blocks
SLOTS = 8            # query blocks per core
CB = C // P          # 8 channel blocks = head pairs
SCALE = 1.0 / np.sqrt(DK)
BF16 = mybir.dt.bfloat16
F32 = mybir.dt.float32
EXP = mybir.ActivationFunctionType.Exp

QBLKS = [
    [0, 2, 4, 6, 9, 11, 13, 15],
    [1, 3, 5, 7, 8, 10, 12, 14],
]

# interleaved jb order: evens ACT load per group and matches attnv's
# half-1 read order so expS slot releases track allocations
ILV = [0, 8, 1, 9, 2, 10, 3, 11, 4, 12, 5, 13, 6, 14, 7, 15]

_cache = {}


def _pop(gen, n):
    for _ in range(n):
        try:
            next(gen)()
        except StopIteration:
            return


def _build():
    nc = bacc.Bacc("TRN2", target_bir_lowering=False, debug=False)

    xT = nc.dram_tensor("xT", [C, T], BF16, kind="ExternalInput").ap()
    xTq = nc.dram_tensor("xTq", [C, SLOTS * P], BF16, kind="ExternalInput").ap()
    wqT = nc.dram_tensor("wqT", [C, C], BF16, kind="ExternalInput").ap()
    wkT = nc.dram_tensor("wkT", [C, C], BF16, kind="ExternalInput").ap()
    wvT = nc.dram_tensor("wvT", [C, C], BF16, kind="ExternalInput").ap()
    woT = nc.dram_tensor("woT", [C, C], BF16, kind="ExternalInput").ap()
    bq = nc.dram_tensor("bq", [P, CB], F32, kind="ExternalInput").ap()
    bk = nc.dram_tensor("bk", [P, CB], F32, kind="ExternalInput").ap()
    masks = nc.dram_tensor("masks", [SLOTS, 2, P, P], BF16, kind="ExternalInput").ap()
    hm2 = nc.dram_tensor("hm2", [1, 2, P], BF16, kind="ExternalInput").ap()
    y = nc.dram_tensor("y", [SLOTS * P, C], F32, kind="ExternalOutput").ap()

    def qchunks(jb):
        """score q-chunks for key block jb: [q0,512) and/or [512,1024)."""
        q0 = P * (jb // 2)
        out = []
        if q0 < 512:
            out.append((q0, 512))
            out.append((512, 1024))
        else:
            out.append((q0, 1024))
        return out

    with tile.TileContext(nc) as tc:
        with (
            tc.tile_pool(name="const", bufs=1) as cpool,
            tc.tile_pool(name="big", bufs=1) as bigpool,
        ):
            hm2_sb = cpool.tile([1, 2, P], BF16)

            v = bigpool.tile([P, NB, H, DK + 1], BF16)
            aT = {
                c: bigpool.tile([P, SLOTS * P], BF16, tag=f"aT{c}",
                                name=f"aT{c}")
                for c in range(CB)
            }
            vg = v[:]
            nc.gpsimd.dma_start(hm2_sb[:], hm2[:])
            nc.vector.memset(vg[:, :, :, DK : DK + 1], 1.0)

            # ---- outer loop pools (survive into the O phase) ----
            with (
                tc.tile_pool(name="expS", bufs=18) as spool,
                tc.tile_pool(name="osb", bufs=2) as osbp,
                tc.tile_pool(name="dnp", bufs=1) as dnp,
                tc.tile_pool(name="rsb", bufs=1) as rsbp,
                tc.tile_pool(name="ps_av", bufs=1, space="PSUM") as ps_av,
                tc.tile_pool(name="ps_r", bufs=1, space="PSUM") as ps_r,
                tc.tile_pool(name="xbig", bufs=1) as xbig,
            ):
                masks_sb = xbig.tile([P, SLOTS, 2, P], BF16)
                bq_sb = xbig.tile([P, CB], F32)
                bk_sb = xbig.tile([P, CB], F32)
                xT_sb = xbig.tile([P, CB, T], BF16)
                xTq_sb = xbig.tile([P, CB, SLOTS * P], BF16)

                # xT first: phase V is the first PE consumer
                nc.gpsimd.dma_start(bq_sb[:], bq[:])
                nc.gpsimd.dma_start(bk_sb[:], bk[:])

                # ---- main loop pools (entered early; phase V borrows
                # their slots, so no pool-close barrier gates tick 1) ----
                inner2 = [
                    tc.tile_pool(name="wqk", bufs=2),
                    tc.tile_pool(name="kq", bufs=2),
                    tc.tile_pool(name="ps_sc", bufs=2, space="PSUM"),
                    tc.tile_pool(name="ps_pj", bufs=1, space="PSUM"),
                ]
                wqk = inner2[0].__enter__()
                kq = inner2[1].__enter__()
                ps_sc = inner2[2].__enter__()
                ps_pj = inner2[3].__enter__()
                kT = {}
                qT = {}
                expS = {}

                def dma_weights(c):
                    wq_t = wqk.tile([P, CB, P], BF16, tag="wq", name=f"wq{c}")
                    wk_t = wqk.tile([P, CB, P], BF16, tag="wk", name=f"wk{c}")
                    nc.gpsimd.dma_start(
                        wq_t[:],
                        wqT.rearrange("(ko p) n -> p ko n", p=P)[
                            :, :, ts(c, P)
                        ],
                    )
                    nc.gpsimd.dma_start(
                        wk_t[:],
                        wkT.rearrange("(ko p) n -> p ko n", p=P)[
                            :, :, ts(c, P)
                        ],
                    )
                    return wq_t, wk_t

                # ---- phase V: value projection, all heads ----
                # wv staged in 4 idle expS slots; accumulators in sc slots
                wvr = wvT.rearrange("(ko p) n -> p ko n", p=P)
                xTr = xT.rearrange("(ko p) t -> p ko t", p=P)
                wv_q = []
                for i in range(4):
                    w_t = spool.tile([P, 2, C], BF16, tag="expS", name=f"wvq{i}")
                    nc.gpsimd.dma_start(w_t[:], wvr[:, 2 * i : 2 * i + 2, :])
                    if i == 0:
                        nc.gpsimd.dma_start(
                            xT_sb[:, :, 0:512], xTr[:, :, 0:512]
                        )
                    wv_q.append(w_t)
                nc.gpsimd.dma_start(xT_sb[:, :, 512:1024], xTr[:, :, 512:1024])
                nc.gpsimd.dma_start(xT_sb[:, :, 1024:T], xTr[:, :, 1024:T])
                nc.gpsimd.dma_start(
                    xTq_sb[:], xTq.rearrange("(ko p) t -> p ko t", p=P)
                )
                nc.gpsimd.dma_start(
                    masks_sb[:], masks[:].rearrange("s t p q -> p s t q")
                )
                wtiles = {0: dma_weights(0)}
                for tb in range(NB):
                    acc = ps_sc.tile([P, 2, 512], F32, tag="sc", name=f"vacc{tb}")
                    for kb in range(CB):
                        for dch in range(2):
                            nc.tensor.matmul(
                                acc[:, dch, :],
                                xT_sb[:, kb, ts(tb, P)],
                                wv_q[kb // 2][:, kb % 2, ts(dch, 512)],
                                start=(kb == 0),
                                stop=(kb == CB - 1),
                            )
                    for dch in range(2):
                        eng = nc.scalar.copy if dch == 0 else (
                            nc.vector.tensor_copy)
                        eng(
                            vg[:, tb, dch * 8 : (dch + 1) * 8, 0:DK],
                            acc[:, dch, :].rearrange("p (h e) -> p h e", e=DK),
                        )

                def qk_units(c):
                    """Generator of PE units for pair-c Q/K projection."""
                    wq_t, wk_t = wtiles[c]
                    qT[c] = kq.tile([P, SLOTS * P], BF16, tag="qT", name=f"qT{c}")
                    kT[c] = kq.tile([P, T], BF16, tag="kT", name=f"kT{c}")
                    for dst, w_t, src_t, nnch, bias in (
                        (qT[c], wq_t, xTq_sb, 2, bq_sb),
                        (kT[c], wk_t, xT_sb, 4, bk_sb),
                    ):
                        for nch in range(nnch):
                            acc = ps_pj.tile([P, 512], F32, tag="pj")
                            for kb in range(CB):
                                def mm(kb=kb, acc=acc, w_t=w_t, src_t=src_t, nch=nch):
                                    nc.tensor.matmul(
                                        acc[:],
                                        w_t[:, kb, :],
                                        src_t[:, kb, ts(nch, 512)],
                                        start=(kb == 0),
                                        stop=(kb == CB - 1),
                                    )
                                yield mm
                            def drain(acc=acc, dst=dst, nch=nch, bias=bias):
                                nc.vector.tensor_scalar_add(
                                    dst[:, ts(nch, 512)], acc[:], bias[:, c : c + 1]
                                )
                            yield drain

                def sc_groups(c):
                    """List of per-jb closures: scores (row-tiled) + exp + mask."""
                    expS[c] = [
                        spool.tile([P, 2, SLOTS * P], BF16, tag="expS",
                                   name=f"expS{c}_{j}")
                        for j in range(NB)
                    ]
                    groups = []
                    for jb in ILV:
                        def grp(jb=jb, c=c):
                            sm = jb // 2
                            for ci, (qa, qb) in enumerate(qchunks(jb)):
                                w = qb - qa
                                pss = ps_sc.tile([P, 2, 512], F32, tag="sc")
                                for h in range(2):
                                    nc.tensor.matmul(
                                        pss[:, h, 0:w],
                                        kT[c][h * DK : (h + 1) * DK, ts(jb, P)],
                                        qT[c][h * DK : (h + 1) * DK, qa:qb],
                                        start=True,
                                        stop=True,
                                    )
                                nc.scalar.activation(
                                    expS[c][jb][:, :, qa:qb],
                                    pss[:, :, 0:w],
                                    EXP,
                                    scale=float(SCALE),
                                )
                                if ci == 0:
                                    for h in range(2):
                                        nc.vector.tensor_mul(
                                            expS[c][jb][:, h, ts(sm, P)],
                                            expS[c][jb][:, h, ts(sm, P)],
                                            masks_sb[:, sm, jb % 2, :],
                                        )
                        groups.append(grp)
                    return groups

                def attnv_units(c):
                    """Generator of PE units for pair-c attnv + normalization."""
                    for half in range(2):
                        qlo, qhi = half * 512, half * 512 + 512
                        pso = ps_av.tile([DK + 1, 2, 512], F32, tag="av")
                        jbs = list(range(8)) if half == 0 else list(ILV)
                        for idx, jb in enumerate(jbs):
                            q0 = P * (jb // 2)
                            qa = max(q0, qlo)
                            for h in range(2):
                                def mm(jb=jb, h=h, qa=qa, pso=pso, idx=idx,
                                       c=c, qlo=qlo, qhi=qhi, last=(idx == len(jbs) - 1)):
                                    nc.tensor.matmul(
                                        pso[:, h, qa - qlo : qhi - qlo],
                                        v[:, jb, 2 * c + h, :],
                                        expS[c][jb][:, h, qa:qhi],
                                        start=(idx == 0),
                                        stop=last,
                                    )
                                yield mm
                        def drain(pso=pso, half=half, qlo=qlo, c=c):
                            # stage O^T per-head to partitions 0:64/64:128 and
                            # the denom row to sbuf, releasing the psum banks
                            osb = osbp.tile([P, 512], BF16, tag="osb")
                            den = dnp.tile([1, 2, 512], BF16, tag="den")
                            nc.scalar.copy(den[:], pso[DK : DK + 1, :, :])
                            nc.vector.tensor_copy(osb[0:DK, :], pso[0:DK, 0, :])
                            nc.scalar.copy(osb[DK:P, :], pso[0:DK, 1, :])
                            # broadcast denoms (rows 0:64 <- head0, 64:128 <- head1)
                            psr = ps_r.tile([P, 512], F32, tag="r")
                            for h in range(2):
                                nc.tensor.matmul(
                                    psr[:],
                                    hm2_sb[0:1, h, :],
                                    den[0:1, h, :],
                                    start=(h == 0),
                                    stop=(h == 1),
                                )
                            r_sb = rsbp.tile([P, 512], F32, tag="rsb")
                            nc.vector.reciprocal_approx_fast(r_sb[:], psr[:])
                            for h in range(2):
                                nc.vector.tensor_mul(
                                    aT[c][h * DK : (h + 1) * DK, qlo : qlo + 512],
                                    osb[h * DK : (h + 1) * DK, :],
                                    r_sb[h * DK : (h + 1) * DK, :],
                                )
                        yield drain

                # ---- pipeline schedule ----
                POPS_AV = [7, 7, 6, 6, 5, 5, 5, 5, 2, 2, 2, 2, 1, 1, 0, 0]
                POPS_QK = [4, 4, 4, 4, 4, 4, 3, 3, 3, 3, 3, 3, 2, 2, 2, 2]

                def tick(sc_c, qk_c, av_c):
                    """One pipeline tick: interleave pair-sc_c scores with
                    pair-qk_c projections and pair-av_c attnv on the PE queue.
                    attnv pops are front-loaded so expS slot releases always
                    precede the exp writes that reuse them (no sem cycle)."""
                    sgs = sc_groups(sc_c)
                    qg = iter(qk_units(qk_c)) if qk_c is not None else iter(())
                    ag = iter(attnv_units(av_c)) if av_c is not None else iter(())
                    for g in range(NB):
                        sgs[g]()
                        _pop(qg, POPS_QK[g])
                        _pop(ag, POPS_AV[g])
                    _pop(qg, 99)
                    _pop(ag, 99)

                # tick 1: QK(0) dense; prefetch w(1)
                for u in qk_units(0):
                    u()
                wtiles[1] = dma_weights(1)
                # tick 2: QK(1) x sc(0)
                tick(0, 1, None)
                wtiles[2] = dma_weights(2)
                # ticks 3..8: QK(c+2) x sc(c+1) x attnv(c)
                for c in range(6):
                    tick(c + 1, c + 2, c)
                    if c + 3 < CB:
                        wtiles[c + 3] = dma_weights(c + 3)
                # tick 9: sc(7) x attnv(6)
                tick(7, None, 6)
                for pool in reversed(inner2):
                    pool.__exit__(None, None, None)

                # ---- output projection, overlapped with attnv(7) ----
                with (
                    tc.tile_pool(name="out", bufs=1) as opool,
                    tc.tile_pool(name="yt", bufs=2) as ytp,
                    tc.tile_pool(name="ps_y", bufs=2, space="PSUM") as ps_y,
                ):
                    wor = woT.rearrange("(ko p) n -> p ko n", p=P)
                    woT_h = []
                    for nch in range(2):
                        w_t = opool.tile([P, CB, 512], BF16, tag=f"wo{nch}",
                                         name=f"wo{nch}")
                        nc.gpsimd.dma_start(w_t[:], wor[:, :, ts(nch, 512)])
                        woT_h.append(w_t)

                    def o_units():
                        for tb in range(SLOTS):
                            y_t = ytp.tile([P, C], F32, tag="yt", name=f"y{tb}")
                            for nch in range(2):
                                psy = ps_y.tile([P, 512], F32, tag="ps_y")
                                for cbk in range(CB):
                                    def mm(psy=psy, cbk=cbk, tb=tb, nch=nch):
                                        nc.tensor.matmul(
                                            psy[:],
                                            aT[cbk][:, ts(tb, P)],
                                            woT_h[nch][:, cbk, :],
                                            start=(cbk == 0),
                                            stop=(cbk == CB - 1),
                                        )
                                    yield mm
                                def drain(psy=psy, y_t=y_t, nch=nch):
                                    nc.vector.tensor_copy(y_t[:, ts(nch, 512)], psy[:])
                                yield drain
                            def dma(y_t=y_t, tb=tb):
                                nc.gpsimd.dma_start(
                                    y.rearrange("(tb p) c -> p tb c", p=P)[:, tb, :],
                                    y_t[:],
                                )
                            yield dma

                    ag7 = iter(attnv_units(7))
                    og = iter(o_units())
                    _pop(ag7, 17)   # half0 MMs + its norm drain first
                    for _ in range(33):
                        _pop(ag7, 1)
                        _pop(og, 2)
                    _pop(og, 999)

    nc.compile()
    return nc


def _host_inputs(x, mask, Wq, bq_v, Wk, bk_v, Wv, bv_v, Wo, bo_v):
    """Per-core input maps + the host-side output bias correction."""
    f32 = np.float32
    bf16 = ml_dtypes.bfloat16
    wqT = np.ascontiguousarray(np.asarray(Wq, f32).T).astype(bf16)
    wkT = np.ascontiguousarray(np.asarray(Wk, f32).T).astype(bf16)
    wvT = np.ascontiguousarray(np.asarray(Wv, f32).T).astype(bf16)
    woT = np.ascontiguousarray(np.asarray(Wo, f32).T).astype(bf16)
    bq_p = np.ascontiguousarray(np.asarray(bq_v, f32).reshape(C // P, P).T)
    bk_p = np.ascontiguousarray(np.asarray(bk_v, f32).reshape(C // P, P).T)
    # exact v/o bias fold: softmax rows sum to 1, so v+bv adds bv to attn out
    bo_eff = (np.asarray(bo_v, f32) + np.asarray(bv_v, f32) @ np.asarray(Wo, f32).T)
    # hm2: rank-2 broadcast matrix for per-head reciprocal rows
    hm2_np = np.zeros((1, 2, P), f32)
    hm2_np[0, 0, 0:DK] = 1.0
    hm2_np[0, 1, DK:P] = 1.0
    hm2_np = hm2_np.astype(bf16)

    # per-half causal boundary masks for the last two key blocks of each slot
    mask_half = []
    tri = np.tril(np.ones((P, P), f32)).T  # [j, i] = 1 where j <= i
    for half in range(2):
        m = np.zeros((SLOTS, 2, P, P), f32)
        for s in range(SLOTS):
            g = QBLKS[half][s]
            for idx, jb in enumerate((2 * s, 2 * s + 1)):
                if jb < g:
                    m[s, idx] = 1.0
                elif jb == g:
                    m[s, idx] = tri
        mask_half.append(m.astype(bf16))

    xn = np.asarray(x, f32)
    in_maps = []
    for core in range(8):
        b, half = divmod(core, 2)
        xT = np.ascontiguousarray(xn[b].T).astype(bf16)
        qtok = np.concatenate([np.arange(g * P, (g + 1) * P) for g in QBLKS[half]])
        xTq = np.ascontiguousarray(xn[b][qtok].T).astype(bf16)
        in_maps.append(
            {
                "xT": xT,
                "xTq": xTq,
                "wqT": wqT,
                "wkT": wkT,
                "wvT": wvT,
                "woT": woT,
                "bq": bq_p,
                "bk": bk_p,
                "masks": mask_half[half],
                "hm2": hm2_np,
            }
        )
    return in_maps, bo_eff


def _run(inputs, trace=False):
    if "nc" not in _cache:
        _cache["nc"] = _build()
    nc = _cache["nc"]
    in_maps, bo_eff = _host_inputs(
        inputs["x"], inputs["mask"],
        inputs["Wq"], inputs["bq"], inputs["Wk"], inputs["bk"],
        inputs["Wv"], inputs["bv"], inputs["Wo"], inputs["bo"],
    )
    res = run_bass_kernel_spmd(nc, in_maps, list(range(8)), trace=trace)
    out = np.empty((B, T, C), np.float32)
    for core in range(8):
        b, half = divmod(core, 2)
        yc = res.results[core]["y"]
        for s, g in enumerate(QBLKS[half]):
            out[b, g * P : (g + 1) * P] = yc[s * P : (s + 1) * P]
    out += bo_eff
    return out, res


def kernel(**inputs):
    out, _ = _run(inputs, trace=False)
    return out


# revision 6
# speedup vs baseline: 1.0206x; 1.0022x over previous
"""Multi-head causal attention (B=4, T=2048, C=1024, H=16) on 8 trn2 cores.

Sharding: data-parallel over batch (4) x sequence-parallel over causal query
blocks (2), zig-zag balanced: core = 2*b + half; half 0 gets query blocks
[0,2,4,6,9,11,13,15], half 1 gets [1,3,5,7,8,10,12,14]. Slot s processes
key blocks 0..2s+1; causal boundary via per-core input masks.

Kernel structure (v2): software-pipelined over head pairs c=0..7.
  tick: [Q/K proj pair c+1] x [scores+exp+mask pair c] x [attnv pair c-1]
Scores are row-tiled (K=64, two heads in PE row strips 0/64 concurrently).
attnv is v-stationary (v tile [128,65] incl ones column -> softmax denom in
psum row 64), streaming wide query chunks; output lands as O^T = aT layout
(no transpose phase). Normalization: reciprocal of denom row + rank-2
broadcast matmul (hm2) + partition-shifted DVE multiplies into aT.
"""

import numpy as np
import ml_dtypes

import concourse.bass as bass
import concourse.mybir as mybir
import concourse.tile as tile
from concourse import bacc
from concourse.bass import ts
from concourse.bass_utils import run_bass_kernel_spmd

B, T, C, H, DK = 4, 2048, 1024, 16, 64
P = 128
NB = T // P          # 16 key blocks
SLOTS = 8            # query blocks per core
CB = C // P          # 8 channel blocks = head pairs
SCALE = 1.0 / np.sqrt(DK)
BF16 = mybir.dt.bfloat16
F32 = mybir.dt.float32
EXP = mybir.ActivationFunctionType.Exp

QBLKS = [
    [0, 2, 4, 6, 9, 11, 13, 15],
    [1, 3, 5, 7, 8, 10, 12, 14],
]

# interleaved jb order: evens ACT load per group and matches attnv's
# half-1 read order so expS slot releases track allocations
ILV = [0, 8, 1, 9, 2, 10, 3, 11, 4, 12, 5, 13, 6, 14, 7, 15]

_cache = {}


def _pop(gen, n):
    for _ in range(n):
        try:
            next(gen)()
        except StopIteration:
            return


def _build():
    nc = bacc.Bacc("TRN2", target_bir_lowering=False, debug=False)

    xT = nc.dram_tensor("xT", [C, T], BF16, kind="ExternalInput").ap()
    xTq = nc.dram_tensor("xTq", [C, SLOTS * P], BF16, kind="ExternalInput").ap()
    wqT = nc.dram_tensor("wqT", [C, C], BF16, kind="ExternalInput").ap()
    wkT = nc.dram_tensor("wkT", [C, C], BF16, kind="ExternalInput").ap()
    wvT = nc.dram_tensor("wvT", [C, C], BF16, kind="ExternalInput").ap()
    woT = nc.dram_tensor("woT", [C, C], BF16, kind="ExternalInput").ap()
    bq = nc.dram_tensor("bq", [P, CB], F32, kind="ExternalInput").ap()
    bk = nc.dram_tensor("bk", [P, CB], F32, kind="ExternalInput").ap()
    masks = nc.dram_tensor("masks", [SLOTS, 2, P, P], BF16, kind="ExternalInput").ap()
    hm2 = nc.dram_tensor("hm2", [1, 2, P], BF16, kind="ExternalInput").ap()
    y = nc.dram_tensor("y", [SLOTS * P, C], F32, kind="ExternalOutput").ap()

    def qchunks(jb):
        """score q-chunks for key block jb: [q0,512) and/or [512,1024)."""
        q0 = P * (jb // 2)
        out = []
        if q0 < 512:
            out.append((q0, 512))
            out.append((512, 1024))
        else:
            out.append((q0, 1024))
        return out

    with tile.TileContext(nc) as tc:
        with (
            tc.tile_pool(name="const", bufs=1) as cpool,
            tc.tile_pool(name="big", bufs=1) as bigpool,
        ):
            hm2_sb = cpool.tile([1, 2, P], BF16)

            v = bigpool.tile([P, NB, H, DK + 1], BF16)
            aT = {
                c: bigpool.tile([P, SLOTS * P], BF16, tag=f"aT{c}",
                                name=f"aT{c}")
                for c in range(CB)
            }
            vg = v[:]
            nc.gpsimd.dma_start(hm2_sb[:], hm2[:])
            nc.vector.memset(vg[:, :, :, DK : DK + 1], 1.0)

            # ---- outer loop pools (survive into the O phase) ----
            with (
                tc.tile_pool(name="expS", bufs=18) as spool,
                tc.tile_pool(name="osb", bufs=2) as osbp,
                tc.tile_pool(name="dnp", bufs=1) as dnp,
                tc.tile_pool(name="rsb", bufs=1) as rsbp,
                tc.tile_pool(name="ps_av", bufs=1, space="PSUM") as ps_av,
                tc.tile_pool(name="ps_r", bufs=1, space="PSUM") as ps_r,
                tc.tile_pool(name="xbig", bufs=1) as xbig,
            ):
                masks_sb = xbig.tile([P, SLOTS, 2, P], BF16)
                bq_sb = xbig.tile([P, CB], F32)
                bk_sb = xbig.tile([P, CB], F32)
                xT_sb = xbig.tile([P, CB, T], BF16)
                xTq_sb = xbig.tile([P, CB, SLOTS * P], BF16)

                # xT first: phase V is the first PE consumer
                nc.gpsimd.dma_start(bq_sb[:], bq[:])
                nc.gpsimd.dma_start(bk_sb[:], bk[:])

                # ---- main loop pools (entered early; phase V borrows
                # their slots, so no pool-close barrier gates tick 1) ----
                inner2 = [
                    tc.tile_pool(name="wqk", bufs=2),
                    tc.tile_pool(name="kq", bufs=2),
                    tc.tile_pool(name="ps_sc", bufs=2, space="PSUM"),
                    tc.tile_pool(name="ps_pj", bufs=1, space="PSUM"),
                ]
                wqk = inner2[0].__enter__()
                kq = inner2[1].__enter__()
                ps_sc = inner2[2].__enter__()
                ps_pj = inner2[3].__enter__()
                kT = {}
                qT = {}
                expS = {}

                def dma_weights(c):
                    wq_t = wqk.tile([P, CB, P], BF16, tag="wq", name=f"wq{c}")
                    wk_t = wqk.tile([P, CB, P], BF16, tag="wk", name=f"wk{c}")
                    nc.gpsimd.dma_start(
                        wq_t[:],
                        wqT.rearrange("(ko p) n -> p ko n", p=P)[
                            :, :, ts(c, P)
                        ],
                    )
                    nc.gpsimd.dma_start(
                        wk_t[:],
                        wkT.rearrange("(ko p) n -> p ko n", p=P)[
                            :, :, ts(c, P)
                        ],
                    )
                    return wq_t, wk_t

                # ---- phase V: value projection, all heads ----
                # wv staged in 4 idle expS slots; accumulators in sc slots
                wvr = wvT.rearrange("(ko p) n -> p ko n", p=P)
                xTr = xT.rearrange("(ko p) t -> p ko t", p=P)
                wv_q = []
                for i in range(4):
                    w_t = spool.tile([P, 2, C], BF16, tag="expS", name=f"wvq{i}")
                    nc.gpsimd.dma_start(w_t[:], wvr[:, 2 * i : 2 * i + 2, :])
                    if i == 0:
                        nc.gpsimd.dma_start(
                            xT_sb[:, :, 0:512], xTr[:, :, 0:512]
                        )
                    wv_q.append(w_t)
                nc.gpsimd.dma_start(xT_sb[:, :, 512:1024], xTr[:, :, 512:1024])
                nc.gpsimd.dma_start(xT_sb[:, :, 1024:T], xTr[:, :, 1024:T])
                nc.gpsimd.dma_start(
                    xTq_sb[:], xTq.rearrange("(ko p) t -> p ko t", p=P)
                )
                nc.gpsimd.dma_start(
                    masks_sb[:], masks[:].rearrange("s t p q -> p s t q")
                )
                wtiles = {0: dma_weights(0)}
                for tb in range(NB):
                    acc = ps_sc.tile([P, 2, 512], F32, tag="sc", name=f"vacc{tb}")
                    for kb in range(CB):
                        for dch in range(2):
                            nc.tensor.matmul(
                                acc[:, dch, :],
                                xT_sb[:, kb, ts(tb, P)],
                                wv_q[kb // 2][:, kb % 2, ts(dch, 512)],
                                start=(kb == 0),
                                stop=(kb == CB - 1),
                            )
                    for dch in range(2):
                        eng = nc.scalar.copy if dch == 0 else (
                            nc.vector.tensor_copy)
                        eng(
                            vg[:, tb, dch * 8 : (dch + 1) * 8, 0:DK],
                            acc[:, dch, :].rearrange("p (h e) -> p h e", e=DK),
                        )

                def qk_units(c):
                    """Generator of PE units for pair-c Q/K projection."""
                    wq_t, wk_t = wtiles[c]
                    qT[c] = kq.tile([P, SLOTS * P], BF16, tag="qT", name=f"qT{c}")
                    kT[c] = kq.tile([P, T], BF16, tag="kT", name=f"kT{c}")
                    for dst, w_t, src_t, nnch, bias in (
                        (qT[c], wq_t, xTq_sb, 2, bq_sb),
                        (kT[c], wk_t, xT_sb, 4, bk_sb),
                    ):
                        for nch in range(nnch):
                            acc = ps_pj.tile([P, 512], F32, tag="pj")
                            for kb in range(CB):
                                def mm(kb=kb, acc=acc, w_t=w_t, src_t=src_t, nch=nch):
                                    nc.tensor.matmul(
                                        acc[:],
                                        w_t[:, kb, :],
                                        src_t[:, kb, ts(nch, 512)],
                                        start=(kb == 0),
                                        stop=(kb == CB - 1),
                                    )
                                yield mm
                            def drain(acc=acc, dst=dst, nch=nch, bias=bias):
                                nc.vector.tensor_scalar_add(
                                    dst[:, ts(nch, 512)], acc[:], bias[:, c : c + 1]
                                )
                            yield drain

                def sc_groups(c):
                    """List of per-jb closures: scores (row-tiled) + exp + mask."""
                    expS[c] = [
                        spool.tile([P, 2, SLOTS * P], BF16, tag="expS",
                                   name=f"expS{c}_{j}")
                        for j in range(NB)
                    ]
                    groups = []
                    for jb in ILV:
                        def grp(jb=jb, c=c):
                            sm = jb // 2
                            for ci, (qa, qb) in enumerate(qchunks(jb)):
                                w = qb - qa
                                pss = ps_sc.tile([P, 2, 512], F32, tag="sc")
                                for h in range(2):
                                    nc.tensor.matmul(
                                        pss[:, h, 0:w],
                                        kT[c][h * DK : (h + 1) * DK, ts(jb, P)],
                                        qT[c][h * DK : (h + 1) * DK, qa:qb],
                                        start=True,
                                        stop=True,
                                    )
                                nc.scalar.activation(
                                    expS[c][jb][:, :, qa:qb],
                                    pss[:, :, 0:w],
                                    EXP,
                                    scale=float(SCALE),
                                )
                                if ci == 0:
                                    for h in range(2):
                                        nc.vector.tensor_mul(
                                            expS[c][jb][:, h, ts(sm, P)],
                                            expS[c][jb][:, h, ts(sm, P)],
                                            masks_sb[:, sm, jb % 2, :],
                                        )
                        groups.append(grp)
                    return groups

                def attnv_units(c):
                    """Generator of PE units for pair-c attnv + normalization."""
                    for half in range(2):
                        qlo, qhi = half * 512, half * 512 + 512
                        pso = ps_av.tile([DK + 1, 2, 512], F32, tag="av")
                        jbs = list(range(8)) if half == 0 else list(ILV)
                        for idx, jb in enumerate(jbs):
                            q0 = P * (jb // 2)
                            qa = max(q0, qlo)
                            for h in range(2):
                                def mm(jb=jb, h=h, qa=qa, pso=pso, idx=idx,
                                       c=c, qlo=qlo, qhi=qhi, last=(idx == len(jbs) - 1)):
                                    nc.tensor.matmul(
                                        pso[:, h, qa - qlo : qhi - qlo],
                                        v[:, jb, 2 * c + h, :],
                                        expS[c][jb][:, h, qa:qhi],
                                        start=(idx == 0),
                                        stop=last,
                                    )
                                yield mm
                        def drain(pso=pso, half=half, qlo=qlo, c=c):
                            # stage O^T per-head to partitions 0:64/64:128 and
                            # the denom row to sbuf, releasing the psum banks
                            osb = osbp.tile([P, 512], BF16, tag="osb")
                            den = dnp.tile([1, 2, 512], BF16, tag="den")
                            nc.scalar.copy(den[:], pso[DK : DK + 1, :, :])
                            nc.vector.tensor_copy(osb[0:DK, :], pso[0:DK, 0, :])
                            nc.scalar.copy(osb[DK:P, :], pso[0:DK, 1, :])
                            # broadcast denoms (rows 0:64 <- head0, 64:128 <- head1)
                            psr = ps_r.tile([P, 512], F32, tag="r")
                            for h in range(2):
                                nc.tensor.matmul(
                                    psr[:],
                                    hm2_sb[0:1, h, :],
                                    den[0:1, h, :],
                                    start=(h == 0),
                                    stop=(h == 1),
                                )
                            r_sb = rsbp.tile([P, 512], F32, tag="rsb")
                            nc.vector.reciprocal_approx_fast(r_sb[:], psr[:])
                            for h in range(2):
                                nc.vector.tensor_mul(
                                    aT[c][h * DK : (h + 1) * DK, qlo : qlo + 512],
                                    osb[h * DK : (h + 1) * DK, :],
                                    r_sb[h * DK : (h + 1) * DK, :],
                                )
                        yield drain

                # ---- pipeline schedule ----
                POPS_AV = [7, 7, 6, 6, 5, 5, 5, 5, 2, 2, 2, 2, 1, 1, 0, 0]
                POPS_QK = [4, 4, 4, 4, 4, 4, 3, 3, 3, 3, 3, 3, 2, 2, 2, 2]

                # qk(7) holds back ~12 units that carry into tick 9
                # (otherwise PE-starved: only sc(7) + attnv(6) there)
                POPS_QK_HOLD = [4, 4, 4, 4, 3, 3, 3, 3, 3, 3, 2, 2, 2, 1, 1, 0]
                POPS_QK_TAIL = [3, 2, 2, 1, 1, 1, 1, 1, 0, 0, 0, 0, 0, 0, 0, 0]

                def tick(sc_c, qg, ag, pops_qk, drain_qk=True):
                    """One pipeline tick: interleave pair-sc_c scores with
                    projection and attnv units on the PE queue. attnv pops are
                    front-loaded so expS slot releases always precede the exp
                    writes that reuse them (no sem cycle)."""
                    sgs = sc_groups(sc_c)
                    for g in range(NB):
                        sgs[g]()
                        _pop(qg, pops_qk[g])
                        _pop(ag, POPS_AV[g])
                    if drain_qk:
                        _pop(qg, 99)
                    _pop(ag, 99)

                EMPTY = iter(())
                # tick 1: QK(0) dense; prefetch w(1)
                for u in qk_units(0):
                    u()
                wtiles[1] = dma_weights(1)
                # tick 2: QK(1) x sc(0)
                tick(0, iter(qk_units(1)), EMPTY, POPS_QK)
                wtiles[2] = dma_weights(2)
                # ticks 3..7: QK(c+2) x sc(c+1) x attnv(c)
                for c in range(5):
                    tick(c + 1, iter(qk_units(c + 2)), iter(attnv_units(c)),
                         POPS_QK)
                    if c + 3 < CB:
                        wtiles[c + 3] = dma_weights(c + 3)
                # tick 8: QK(7) partially held back
                qg7 = iter(qk_units(7))
                tick(6, qg7, iter(attnv_units(5)), POPS_QK_HOLD, drain_qk=False)
                # tick 9: sc(7) x attnv(6) x leftover QK(7)
                tick(7, qg7, iter(attnv_units(6)), POPS_QK_TAIL)
                for pool in reversed(inner2):
                    pool.__exit__(None, None, None)

                # ---- output projection, overlapped with attnv(7) ----
                with (
                    tc.tile_pool(name="out", bufs=1) as opool,
                    tc.tile_pool(name="yt", bufs=2) as ytp,
                    tc.tile_pool(name="ps_y", bufs=2, space="PSUM") as ps_y,
                ):
                    wor = woT.rearrange("(ko p) n -> p ko n", p=P)
                    woT_h = []
                    for nch in range(2):
                        w_t = opool.tile([P, CB, 512], BF16, tag=f"wo{nch}",
                                         name=f"wo{nch}")
                        nc.gpsimd.dma_start(w_t[:], wor[:, :, ts(nch, 512)])
                        woT_h.append(w_t)

                    def o_units():
                        for tb in range(SLOTS):
                            y_t = ytp.tile([P, C], F32, tag="yt", name=f"y{tb}")
                            for nch in range(2):
                                psy = ps_y.tile([P, 512], F32, tag="ps_y")
                                for cbk in range(CB):
                                    def mm(psy=psy, cbk=cbk, tb=tb, nch=nch):
                                        nc.tensor.matmul(
                                            psy[:],
                                            aT[cbk][:, ts(tb, P)],
                                            woT_h[nch][:, cbk, :],
                                            start=(cbk == 0),
                                            stop=(cbk == CB - 1),
                                        )
                                    yield mm
                                def drain(psy=psy, y_t=y_t, nch=nch):
                                    nc.vector.tensor_copy(y_t[:, ts(nch, 512)], psy[:])
                                yield drain
                            def dma(y_t=y_t, tb=tb):
                                nc.gpsimd.dma_start(
                                    y.rearrange("(tb p) c -> p tb c", p=P)[:, tb, :],
                                    y_t[:],
                                )
                            yield dma

                    ag7 = iter(attnv_units(7))
                    og = iter(o_units())
                    _pop(ag7, 17)   # half0 MMs + its norm drain first
                    for _ in range(17):
                        _pop(ag7, 2)
                        _pop(og, 2)
                    _pop(og, 999)

    nc.compile()
    return nc


def _host_inputs(x, mask, Wq, bq_v, Wk, bk_v, Wv, bv_v, Wo, bo_v):
    """Per-core input maps + the host-side output bias correction."""
    f32 = np.float32
    bf16 = ml_dtypes.bfloat16
    wqT = np.ascontiguousarray(np.asarray(Wq, f32).T).astype(bf16)
    wkT = np.ascontiguousarray(np.asarray(Wk, f32).T).astype(bf16)
    wvT = np.ascontiguousarray(np.asarray(Wv, f32).T).astype(bf16)
    woT = np.ascontiguousarray(np.asarray(Wo, f32).T).astype(bf16)
    bq_p = np.ascontiguousarray(np.asarray(bq_v, f32).reshape(C // P, P).T)
    bk_p = np.ascontiguousarray(np.asarray(bk_v, f32).reshape(C // P, P).T)
    # exact v/o bias fold: softmax rows sum to 1, so v+bv adds bv to attn out
    bo_eff = (np.asarray(bo_v, f32) + np.asarray(bv_v, f32) @ np.asarray(Wo, f32).T)
    # hm2: rank-2 broadcast matrix for per-head reciprocal rows
    hm2_np = np.zeros((1, 2, P), f32)
    hm2_np[0, 0, 0:DK] = 1.0
    hm2_np[0, 1, DK:P] = 1.0
    hm2_np = hm2_np.astype(bf16)

    # per-half causal boundary masks for the last two key blocks of each slot
    mask_half = []
    tri = np.tril(np.ones((P, P), f32)).T  # [j, i] = 1 where j <= i
    for half in range(2):
        m = np.zeros((SLOTS, 2, P, P), f32)
        for s in range(SLOTS):
            g = QBLKS[half][s]
            for idx, jb in enumerate((2 * s, 2 * s + 1)):
                if jb < g:
                    m[s, idx] = 1.0
                elif jb == g:
                    m[s, idx] = tri
        mask_half.append(m.astype(bf16))

    xn = np.asarray(x, f32)
    in_maps = []
    for core in range(8):
        b, half = divmod(core, 2)
        xT = np.ascontiguousarray(xn[b].T).astype(bf16)
        qtok = np.concatenate([np.arange(g * P, (g + 1) * P) for g in QBLKS[half]])
        xTq = np.ascontiguousarray(xn[b][qtok].T).astype(bf16)
        in_maps.append(
            {
                "xT": xT,
                "xTq": xTq,
                "wqT": wqT,
                "wkT": wkT,
                "wvT": wvT,
                "woT": woT,
                "bq": bq_p,
                "bk": bk_p,
                "masks": mask_half[half],
                "hm2": hm2_np,
            }
        )
    return in_maps, bo_eff


def _run(inputs, trace=False):
    if "nc" not in _cache:
        _cache["nc"] = _build()
    nc = _cache["nc"]
    in_maps, bo_eff = _host_inputs(
        inputs["x"], inputs["mask"],
        inputs["Wq"], inputs["bq"], inputs["Wk"], inputs["bk"],
        inputs["Wv"], inputs["bv"], inputs["Wo"], inputs["bo"],
    )
    res = run_bass_kernel_spmd(nc, in_maps, list(range(8)), trace=trace)
    out = np.empty((B, T, C), np.float32)
    for core in range(8):
        b, half = divmod(core, 2)
        yc = res.results[core]["y"]
        for s, g in enumerate(QBLKS[half]):
            out[b, g * P : (g + 1) * P] = yc[s * P : (s + 1) * P]
    out += bo_eff
    return out, res


def kernel(**inputs):
    out, _ = _run(inputs, trace=False)
    return out


# revision 7
# speedup vs baseline: 1.0210x; 1.0004x over previous
"""Multi-head causal attention (B=4, T=2048, C=1024, H=16) on 8 trn2 cores.

Sharding: data-parallel over batch (4) x sequence-parallel over causal query
blocks (2), zig-zag balanced: core = 2*b + half; half 0 gets query blocks
[0,2,4,6,9,11,13,15], half 1 gets [1,3,5,7,8,10,12,14]. Slot s processes
key blocks 0..2s+1; causal boundary via per-core input masks.

Kernel structure (v2): software-pipelined over head pairs c=0..7.
  tick: [Q/K proj pair c+1] x [scores+exp+mask pair c] x [attnv pair c-1]
Scores are row-tiled (K=64, two heads in PE row strips 0/64 concurrently).
attnv is v-stationary (v tile [128,65] incl ones column -> softmax denom in
psum row 64), streaming wide query chunks; output lands as O^T = aT layout
(no transpose phase). Normalization: reciprocal of denom row + rank-2
broadcast matmul (hm2) + partition-shifted DVE multiplies into aT.
"""

import numpy as np
import ml_dtypes

import concourse.bass as bass
import concourse.mybir as mybir
import concourse.tile as tile
from concourse import bacc
from concourse.bass import ts
from concourse.bass_utils import run_bass_kernel_spmd

B, T, C, H, DK = 4, 2048, 1024, 16, 64
P = 128
NB = T // P          # 16 key blocks
SLOTS = 8            # query blocks per core
CB = C // P          # 8 channel blocks = head pairs
SCALE = 1.0 / np.sqrt(DK)
BF16 = mybir.dt.bfloat16
F32 = mybir.dt.float32
EXP = mybir.ActivationFunctionType.Exp

QBLKS = [
    [0, 2, 4, 6, 9, 11, 13, 15],
    [1, 3, 5, 7, 8, 10, 12, 14],
]

# interleaved jb order: evens ACT load per group and matches attnv's
# half-1 read order so expS slot releases track allocations
ILV = [0, 8, 1, 9, 2, 10, 3, 11, 4, 12, 5, 13, 6, 14, 7, 15]

_cache = {}


def _pop(gen, n):
    for _ in range(n):
        try:
            next(gen)()
        except StopIteration:
            return


def _build():
    nc = bacc.Bacc("TRN2", target_bir_lowering=False, debug=False)

    xT = nc.dram_tensor("xT", [C, T], BF16, kind="ExternalInput").ap()
    xTq = nc.dram_tensor("xTq", [C, SLOTS * P], BF16, kind="ExternalInput").ap()
    wqT = nc.dram_tensor("wqT", [C, C], BF16, kind="ExternalInput").ap()
    wkT = nc.dram_tensor("wkT", [C, C], BF16, kind="ExternalInput").ap()
    wvT = nc.dram_tensor("wvT", [C, C], BF16, kind="ExternalInput").ap()
    woT = nc.dram_tensor("woT", [C, C], BF16, kind="ExternalInput").ap()
    bq = nc.dram_tensor("bq", [P, CB], F32, kind="ExternalInput").ap()
    bk = nc.dram_tensor("bk", [P, CB], F32, kind="ExternalInput").ap()
    masks = nc.dram_tensor("masks", [SLOTS, 2, P, P], BF16, kind="ExternalInput").ap()
    hm2 = nc.dram_tensor("hm2", [1, 2, P], BF16, kind="ExternalInput").ap()
    y = nc.dram_tensor("y", [SLOTS * P, C], F32, kind="ExternalOutput").ap()

    def qchunks(jb):
        """score q-chunks for key block jb: [q0,512) and/or [512,1024)."""
        q0 = P * (jb // 2)
        out = []
        if q0 < 512:
            out.append((q0, 512))
            out.append((512, 1024))
        else:
            out.append((q0, 1024))
        return out

    with tile.TileContext(nc) as tc:
        with (
            tc.tile_pool(name="const", bufs=1) as cpool,
            tc.tile_pool(name="big", bufs=1) as bigpool,
        ):
            hm2_sb = cpool.tile([1, 2, P], BF16)

            v = bigpool.tile([P, NB, H, DK + 1], BF16)
            aT = {
                c: bigpool.tile([P, SLOTS * P], BF16, tag=f"aT{c}",
                                name=f"aT{c}")
                for c in range(CB)
            }
            vg = v[:]
            nc.gpsimd.dma_start(hm2_sb[:], hm2[:])
            nc.vector.memset(vg[:, :, :, DK : DK + 1], 1.0)

            # ---- outer loop pools (survive into the O phase) ----
            with (
                tc.tile_pool(name="expS", bufs=18) as spool,
                tc.tile_pool(name="osb", bufs=2) as osbp,
                tc.tile_pool(name="dnp", bufs=1) as dnp,
                tc.tile_pool(name="rsb", bufs=1) as rsbp,
                tc.tile_pool(name="ps_av", bufs=1, space="PSUM") as ps_av,
                tc.tile_pool(name="ps_r", bufs=1, space="PSUM") as ps_r,
                tc.tile_pool(name="xbig", bufs=1) as xbig,
            ):
                masks_sb = xbig.tile([P, SLOTS, 2, P], BF16)
                bq_sb = xbig.tile([P, CB], F32)
                bk_sb = xbig.tile([P, CB], F32)
                xT_sb = xbig.tile([P, CB, T], BF16)
                xTq_sb = xbig.tile([P, CB, SLOTS * P], BF16)

                # xT first: phase V is the first PE consumer
                nc.gpsimd.dma_start(bq_sb[:], bq[:])
                nc.gpsimd.dma_start(bk_sb[:], bk[:])

                # ---- main loop pools (entered early; phase V borrows
                # their slots, so no pool-close barrier gates tick 1) ----
                inner2 = [
                    tc.tile_pool(name="wqk", bufs=2),
                    tc.tile_pool(name="kq", bufs=2),
                    tc.tile_pool(name="ps_sc", bufs=2, space="PSUM"),
                    tc.tile_pool(name="ps_pj", bufs=1, space="PSUM"),
                ]
                wqk = inner2[0].__enter__()
                kq = inner2[1].__enter__()
                ps_sc = inner2[2].__enter__()
                ps_pj = inner2[3].__enter__()
                kT = {}
                qT = {}
                expS = {}

                def dma_weights(c):
                    wq_t = wqk.tile([P, CB, P], BF16, tag="wq", name=f"wq{c}")
                    wk_t = wqk.tile([P, CB, P], BF16, tag="wk", name=f"wk{c}")
                    nc.gpsimd.dma_start(
                        wq_t[:],
                        wqT.rearrange("(ko p) n -> p ko n", p=P)[
                            :, :, ts(c, P)
                        ],
                    )
                    nc.gpsimd.dma_start(
                        wk_t[:],
                        wkT.rearrange("(ko p) n -> p ko n", p=P)[
                            :, :, ts(c, P)
                        ],
                    )
                    return wq_t, wk_t

                # ---- phase V: value projection, all heads ----
                # wv staged in 4 idle expS slots; accumulators in sc slots
                wvr = wvT.rearrange("(ko p) n -> p ko n", p=P)
                xTr = xT.rearrange("(ko p) t -> p ko t", p=P)
                wv_q = []
                for i in range(4):
                    w_t = spool.tile([P, 2, C], BF16, tag="expS", name=f"wvq{i}")
                    nc.gpsimd.dma_start(w_t[:], wvr[:, 2 * i : 2 * i + 2, :])
                    if i == 0:
                        nc.gpsimd.dma_start(
                            xT_sb[:, :, 0:512], xTr[:, :, 0:512]
                        )
                    wv_q.append(w_t)
                nc.gpsimd.dma_start(xT_sb[:, :, 512:1024], xTr[:, :, 512:1024])
                nc.gpsimd.dma_start(xT_sb[:, :, 1024:T], xTr[:, :, 1024:T])
                nc.gpsimd.dma_start(
                    xTq_sb[:], xTq.rearrange("(ko p) t -> p ko t", p=P)
                )
                nc.gpsimd.dma_start(
                    masks_sb[:], masks[:].rearrange("s t p q -> p s t q")
                )
                wtiles = {0: dma_weights(0)}
                for tb in range(NB):
                    acc = ps_sc.tile([P, 2, 512], F32, tag="sc", name=f"vacc{tb}")
                    for kb in range(CB):
                        for dch in range(2):
                            nc.tensor.matmul(
                                acc[:, dch, :],
                                xT_sb[:, kb, ts(tb, P)],
                                wv_q[kb // 2][:, kb % 2, ts(dch, 512)],
                                start=(kb == 0),
                                stop=(kb == CB - 1),
                            )
                    for dch in range(2):
                        eng = nc.scalar.copy if dch == 0 else (
                            nc.vector.tensor_copy)
                        eng(
                            vg[:, tb, dch * 8 : (dch + 1) * 8, 0:DK],
                            acc[:, dch, :].rearrange("p (h e) -> p h e", e=DK),
                        )

                def qk_units(c):
                    """Generator of PE units for pair-c Q/K projection."""
                    wq_t, wk_t = wtiles[c]
                    qT[c] = kq.tile([P, SLOTS * P], BF16, tag="qT", name=f"qT{c}")
                    kT[c] = kq.tile([P, T], BF16, tag="kT", name=f"kT{c}")
                    for dst, w_t, src_t, nnch, bias in (
                        (qT[c], wq_t, xTq_sb, 2, bq_sb),
                        (kT[c], wk_t, xT_sb, 4, bk_sb),
                    ):
                        for nch in range(nnch):
                            acc = ps_pj.tile([P, 512], F32, tag="pj")
                            for kb in range(CB):
                                def mm(kb=kb, acc=acc, w_t=w_t, src_t=src_t, nch=nch):
                                    nc.tensor.matmul(
                                        acc[:],
                                        w_t[:, kb, :],
                                        src_t[:, kb, ts(nch, 512)],
                                        start=(kb == 0),
                                        stop=(kb == CB - 1),
                                    )
                                yield mm
                            def drain(acc=acc, dst=dst, nch=nch, bias=bias):
                                nc.vector.tensor_scalar_add(
                                    dst[:, ts(nch, 512)], acc[:], bias[:, c : c + 1]
                                )
                            yield drain

                def sc_groups(c):
                    """List of per-jb closures: scores (row-tiled) + exp + mask."""
                    expS[c] = [
                        spool.tile([P, 2, SLOTS * P], BF16, tag="expS",
                                   name=f"expS{c}_{j}")
                        for j in range(NB)
                    ]
                    groups = []
                    for jb in ILV:
                        def grp(jb=jb, c=c):
                            sm = jb // 2
                            for ci, (qa, qb) in enumerate(qchunks(jb)):
                                w = qb - qa
                                pss = ps_sc.tile([P, 2, 512], F32, tag="sc")
                                for h in range(2):
                                    nc.tensor.matmul(
                                        pss[:, h, 0:w],
                                        kT[c][h * DK : (h + 1) * DK, ts(jb, P)],
                                        qT[c][h * DK : (h + 1) * DK, qa:qb],
                                        start=True,
                                        stop=True,
                                    )
                                nc.scalar.activation(
                                    expS[c][jb][:, :, qa:qb],
                                    pss[:, :, 0:w],
                                    EXP,
                                    scale=float(SCALE),
                                )
                                if ci == 0:
                                    for h in range(2):
                                        nc.vector.tensor_mul(
                                            expS[c][jb][:, h, ts(sm, P)],
                                            expS[c][jb][:, h, ts(sm, P)],
                                            masks_sb[:, sm, jb % 2, :],
                                        )
                        groups.append(grp)
                    return groups

                def attnv_units(c):
                    """Generator of PE units for pair-c attnv + normalization."""
                    for half in range(2):
                        qlo, qhi = half * 512, half * 512 + 512
                        pso = ps_av.tile([DK + 1, 2, 512], F32, tag="av")
                        jbs = list(range(8)) if half == 0 else list(ILV)
                        for idx, jb in enumerate(jbs):
                            q0 = P * (jb // 2)
                            qa = max(q0, qlo)
                            for h in range(2):
                                def mm(jb=jb, h=h, qa=qa, pso=pso, idx=idx,
                                       c=c, qlo=qlo, qhi=qhi, last=(idx == len(jbs) - 1)):
                                    nc.tensor.matmul(
                                        pso[:, h, qa - qlo : qhi - qlo],
                                        v[:, jb, 2 * c + h, :],
                                        expS[c][jb][:, h, qa:qhi],
                                        start=(idx == 0),
                                        stop=last,
                                    )
                                yield mm
                        def drain(pso=pso, half=half, qlo=qlo, c=c):
                            # stage O^T per-head to partitions 0:64/64:128 and
                            # the denom row to sbuf, releasing the psum banks
                            osb = osbp.tile([P, 512], BF16, tag="osb")
                            den = dnp.tile([1, 2, 512], BF16, tag="den")
                            nc.scalar.copy(den[:], pso[DK : DK + 1, :, :])
                            nc.vector.tensor_copy(osb[0:DK, :], pso[0:DK, 0, :])
                            nc.scalar.copy(osb[DK:P, :], pso[0:DK, 1, :])
                            # broadcast denoms (rows 0:64 <- head0, 64:128 <- head1)
                            psr = ps_r.tile([P, 512], F32, tag="r")
                            for h in range(2):
                                nc.tensor.matmul(
                                    psr[:],
                                    hm2_sb[0:1, h, :],
                                    den[0:1, h, :],
                                    start=(h == 0),
                                    stop=(h == 1),
                                )
                            r_sb = rsbp.tile([P, 512], F32, tag="rsb")
                            nc.vector.reciprocal_approx_fast(r_sb[:], psr[:])
                            for h in range(2):
                                nc.vector.tensor_mul(
                                    aT[c][h * DK : (h + 1) * DK, qlo : qlo + 512],
                                    osb[h * DK : (h + 1) * DK, :],
                                    r_sb[h * DK : (h + 1) * DK, :],
                                )
                        yield drain

                # ---- pipeline schedule ----
                POPS_AV = [7, 7, 6, 6, 5, 5, 5, 5, 2, 2, 2, 2, 1, 1, 0, 0]
                POPS_QK = [4, 4, 4, 4, 4, 4, 3, 3, 3, 3, 3, 3, 2, 2, 2, 2]

                # qk(7) holds back ~12 units that carry into tick 9
                # (otherwise PE-starved: only sc(7) + attnv(6) there)
                POPS_QK_HOLD = [4, 4, 4, 4, 3, 3, 3, 3, 3, 3, 2, 2, 2, 1, 1, 0]
                POPS_QK_TAIL = [3, 2, 2, 1, 1, 1, 1, 1, 0, 0, 0, 0, 0, 0, 0, 0]

                def tick(sc_c, qg, ag, pops_qk, drain_qk=True):
                    """One pipeline tick: interleave pair-sc_c scores with
                    projection and attnv units on the PE queue. attnv pops are
                    front-loaded so expS slot releases always precede the exp
                    writes that reuse them (no sem cycle)."""
                    sgs = sc_groups(sc_c)
                    for g in range(NB):
                        sgs[g]()
                        _pop(qg, pops_qk[g])
                        _pop(ag, POPS_AV[g])
                    if drain_qk:
                        _pop(qg, 99)
                    _pop(ag, 99)

                EMPTY = iter(())
                # tick 1: QK(0) dense; prefetch w(1)
                for u in qk_units(0):
                    u()
                wtiles[1] = dma_weights(1)
                # tick 2: QK(1) x sc(0)
                tick(0, iter(qk_units(1)), EMPTY, POPS_QK)
                wtiles[2] = dma_weights(2)
                # ticks 3..7: QK(c+2) x sc(c+1) x attnv(c)
                for c in range(5):
                    tick(c + 1, iter(qk_units(c + 2)), iter(attnv_units(c)),
                         POPS_QK)
                    if c + 3 < CB:
                        wtiles[c + 3] = dma_weights(c + 3)
                # tick 8: QK(7) partially held back
                qg7 = iter(qk_units(7))
                tick(6, qg7, iter(attnv_units(5)), POPS_QK_HOLD, drain_qk=False)
                # tick 9: sc(7) x attnv(6) x leftover QK(7); then attnv(7)
                # half0 (its exps are done by group 14)
                ag7 = iter(attnv_units(7))
                tick(7, qg7, iter(attnv_units(6)), POPS_QK_TAIL)
                _pop(ag7, 17)   # half0 MMs + norm drain
                for pool in reversed(inner2):
                    pool.__exit__(None, None, None)

                # ---- output projection, overlapped with attnv(7) ----
                with (
                    tc.tile_pool(name="out", bufs=1) as opool,
                    tc.tile_pool(name="yt", bufs=2) as ytp,
                    tc.tile_pool(name="ps_y", bufs=2, space="PSUM") as ps_y,
                ):
                    wor = woT.rearrange("(ko p) n -> p ko n", p=P)
                    woT_h = []
                    for nch in range(2):
                        w_t = opool.tile([P, CB, 512], BF16, tag=f"wo{nch}",
                                         name=f"wo{nch}")
                        nc.gpsimd.dma_start(w_t[:], wor[:, :, ts(nch, 512)])
                        woT_h.append(w_t)

                    def o_units():
                        yr = y.rearrange("(tb p) c -> p tb c", p=P)
                        for tb in range(SLOTS):
                            for nch in range(2):
                                y_t = ytp.tile([P, 512], F32, tag="yt",
                                               name=f"y{tb}_{nch}")
                                psy = ps_y.tile([P, 512], F32, tag="ps_y")
                                for cbk in range(CB):
                                    def mm(psy=psy, cbk=cbk, tb=tb, nch=nch):
                                        nc.tensor.matmul(
                                            psy[:],
                                            aT[cbk][:, ts(tb, P)],
                                            woT_h[nch][:, cbk, :],
                                            start=(cbk == 0),
                                            stop=(cbk == CB - 1),
                                        )
                                    yield mm
                                def drain(psy=psy, y_t=y_t, tb=tb, nch=nch):
                                    nc.vector.tensor_copy(y_t[:], psy[:])
                                    nc.gpsimd.dma_start(
                                        yr[:, tb, ts(nch, 512)], y_t[:]
                                    )
                                yield drain

                    og = iter(o_units())
                    for _ in range(17):
                        _pop(ag7, 2)
                        _pop(og, 2)
                    _pop(og, 999)

    nc.compile()
    return nc


def _host_inputs(x, mask, Wq, bq_v, Wk, bk_v, Wv, bv_v, Wo, bo_v):
    """Per-core input maps + the host-side output bias correction."""
    f32 = np.float32
    bf16 = ml_dtypes.bfloat16
    wqT = np.ascontiguousarray(np.asarray(Wq, f32).T).astype(bf16)
    wkT = np.ascontiguousarray(np.asarray(Wk, f32).T).astype(bf16)
    wvT = np.ascontiguousarray(np.asarray(Wv, f32).T).astype(bf16)
    woT = np.ascontiguousarray(np.asarray(Wo, f32).T).astype(bf16)
    bq_p = np.ascontiguousarray(np.asarray(bq_v, f32).reshape(C // P, P).T)
    bk_p = np.ascontiguousarray(np.asarray(bk_v, f32).reshape(C // P, P).T)
    # exact v/o bias fold: softmax rows sum to 1, so v+bv adds bv to attn out
    bo_eff = (np.asarray(bo_v, f32) + np.asarray(bv_v, f32) @ np.asarray(Wo, f32).T)
    # hm2: rank-2 broadcast matrix for per-head reciprocal rows
    hm2_np = np.zeros((1, 2, P), f32)
    hm2_np[0, 0, 0:DK] = 1.0
    hm2_np[0, 1, DK:P] = 1.0
    hm2_np = hm2_np.astype(bf16)

    # per-half causal boundary masks for the last two key blocks of each slot
    mask_half = []
    tri = np.tril(np.ones((P, P), f32)).T  # [j, i] = 1 where j <= i
    for half in range(2):
        m = np.zeros((SLOTS, 2, P, P), f32)
        for s in range(SLOTS):
            g = QBLKS[half][s]
            for idx, jb in enumerate((2 * s, 2 * s + 1)):
                if jb < g:
                    m[s, idx] = 1.0
                elif jb == g:
                    m[s, idx] = tri
        mask_half.append(m.astype(bf16))

    xn = np.asarray(x, f32)
    in_maps = []
    for core in range(8):
        b, half = divmod(core, 2)
        xT = np.ascontiguousarray(xn[b].T).astype(bf16)
        qtok = np.concatenate([np.arange(g * P, (g + 1) * P) for g in QBLKS[half]])
        xTq = np.ascontiguousarray(xn[b][qtok].T).astype(bf16)
        in_maps.append(
            {
                "xT": xT,
                "xTq": xTq,
                "wqT": wqT,
                "wkT": wkT,
                "wvT": wvT,
                "woT": woT,
                "bq": bq_p,
                "bk": bk_p,
                "masks": mask_half[half],
                "hm2": hm2_np,
            }
        )
    return in_maps, bo_eff


def _run(inputs, trace=False):
    if "nc" not in _cache:
        _cache["nc"] = _build()
    nc = _cache["nc"]
    in_maps, bo_eff = _host_inputs(
        inputs["x"], inputs["mask"],
        inputs["Wq"], inputs["bq"], inputs["Wk"], inputs["bk"],
        inputs["Wv"], inputs["bv"], inputs["Wo"], inputs["bo"],
    )
    res = run_bass_kernel_spmd(nc, in_maps, list(range(8)), trace=trace)
    out = np.empty((B, T, C), np.float32)
    for core in range(8):
        b, half = divmod(core, 2)
        yc = res.results[core]["y"]
        for s, g in enumerate(QBLKS[half]):
            out[b, g * P : (g + 1) * P] = yc[s * P : (s + 1) * P]
    out += bo_eff
    return out, res


def kernel(**inputs):
    out, _ = _run(inputs, trace=False)
    return out


# revision 8
# speedup vs baseline: 1.0223x; 1.0013x over previous
"""Multi-head causal attention (B=4, T=2048, C=1024, H=16) on 8 trn2 cores.

Sharding: data-parallel over batch (4) x sequence-parallel over causal query
blocks (2), zig-zag balanced: core = 2*b + half; half 0 gets query blocks
[0,2,4,6,9,11,13,15], half 1 gets [1,3,5,7,8,10,12,14]. Slot s processes
key blocks 0..2s+1; causal boundary via per-core input masks.

Kernel structure (v2): software-pipelined over head pairs c=0..7.
  tick: [Q/K proj pair c+1] x [scores+exp+mask pair c] x [attnv pair c-1]
Scores are row-tiled (K=64, two heads in PE row strips 0/64 concurrently).
attnv is v-stationary (v tile [128,65] incl ones column -> softmax denom in
psum row 64), streaming wide query chunks; output lands as O^T = aT layout
(no transpose phase). Normalization: reciprocal of denom row + rank-2
broadcast matmul (hm2) + partition-shifted DVE multiplies into aT.
"""

import numpy as np
import ml_dtypes

import concourse.bass as bass
import concourse.mybir as mybir
import concourse.tile as tile
from concourse import bacc
from concourse.bass import ts
from concourse.bass_utils import run_bass_kernel_spmd

B, T, C, H, DK = 4, 2048, 1024, 16, 64
P = 128
NB = T // P          # 16 key blocks
SLOTS = 8            # query blocks per core
CB = C // P          # 8 channel blocks = head pairs
SCALE = 1.0 / np.sqrt(DK)
BF16 = mybir.dt.bfloat16
F32 = mybir.dt.float32
EXP = mybir.ActivationFunctionType.Exp

QBLKS = [
    [0, 2, 4, 6, 9, 11, 13, 15],
    [1, 3, 5, 7, 8, 10, 12, 14],
]

# interleaved jb order: evens ACT load per group and matches attnv's
# half-1 read order so expS slot releases track allocations
ILV = [0, 8, 1, 9, 2, 10, 3, 11, 4, 12, 5, 13, 6, 14, 7, 15]

_cache = {}


def _pop(gen, n):
    for _ in range(n):
        try:
            next(gen)()
        except StopIteration:
            return


def _build():
    nc = bacc.Bacc("TRN2", target_bir_lowering=False, debug=False)

    xT = nc.dram_tensor("xT", [C, T], BF16, kind="ExternalInput").ap()
    xTq = nc.dram_tensor("xTq", [C, SLOTS * P], BF16, kind="ExternalInput").ap()
    wqT = nc.dram_tensor("wqT", [C, C], BF16, kind="ExternalInput").ap()
    wkT = nc.dram_tensor("wkT", [C, C], BF16, kind="ExternalInput").ap()
    wvT = nc.dram_tensor("wvT", [C, C], BF16, kind="ExternalInput").ap()
    woT = nc.dram_tensor("woT", [C, C], BF16, kind="ExternalInput").ap()
    bq = nc.dram_tensor("bq", [P, CB], F32, kind="ExternalInput").ap()
    bk = nc.dram_tensor("bk", [P, CB], F32, kind="ExternalInput").ap()
    masks = nc.dram_tensor("masks", [SLOTS, 2, P, P], BF16, kind="ExternalInput").ap()
    hm2 = nc.dram_tensor("hm2", [1, 2, P], BF16, kind="ExternalInput").ap()
    y = nc.dram_tensor("y", [SLOTS * P, C], F32, kind="ExternalOutput").ap()

    def qchunks(jb):
        """score q-chunks for key block jb: [q0,512) and/or [512,1024)."""
        q0 = P * (jb // 2)
        out = []
        if q0 < 512:
            out.append((q0, 512))
            out.append((512, 1024))
        else:
            out.append((q0, 1024))
        return out

    with tile.TileContext(nc) as tc:
        with (
            tc.tile_pool(name="const", bufs=1) as cpool,
            tc.tile_pool(name="big", bufs=1) as bigpool,
        ):
            hm2_sb = cpool.tile([1, 2, P], BF16)

            v = bigpool.tile([P, NB, H, DK + 1], BF16)
            aT = {
                c: bigpool.tile([P, SLOTS * P], BF16, tag=f"aT{c}",
                                name=f"aT{c}")
                for c in range(CB)
            }
            vg = v[:]
            nc.gpsimd.dma_start(hm2_sb[:], hm2[:])
            nc.vector.memset(vg[:, :, :, DK : DK + 1], 1.0)

            # ---- outer loop pools (survive into the O phase) ----
            with (
                tc.tile_pool(name="expS", bufs=18) as spool,
                tc.tile_pool(name="osb", bufs=2) as osbp,
                tc.tile_pool(name="dnp", bufs=1) as dnp,
                tc.tile_pool(name="rsb", bufs=1) as rsbp,
                tc.tile_pool(name="ps_av", bufs=1, space="PSUM") as ps_av,
                tc.tile_pool(name="ps_r", bufs=1, space="PSUM") as ps_r,
                tc.tile_pool(name="xbig", bufs=1) as xbig,
            ):
                masks_sb = xbig.tile([P, SLOTS, 2, P], BF16)
                bq_sb = xbig.tile([P, CB], F32)
                bk_sb = xbig.tile([P, CB], F32)
                xTp = [
                    xbig.tile([P, CB, 512], BF16, tag=f"xTp{i}", name=f"xTp{i}")
                    for i in range(4)
                ]
                xTq_sb = xbig.tile([P, CB, SLOTS * P], BF16)

                # xT first: phase V is the first PE consumer
                nc.gpsimd.dma_start(bq_sb[:], bq[:])
                nc.gpsimd.dma_start(bk_sb[:], bk[:])

                # ---- main loop pools (entered early; phase V borrows
                # their slots, so no pool-close barrier gates tick 1) ----
                inner2 = [
                    tc.tile_pool(name="wqk", bufs=2),
                    tc.tile_pool(name="kq", bufs=2),
                    tc.tile_pool(name="ps_sc", bufs=2, space="PSUM"),
                    tc.tile_pool(name="ps_pj", bufs=1, space="PSUM"),
                ]
                wqk = inner2[0].__enter__()
                kq = inner2[1].__enter__()
                ps_sc = inner2[2].__enter__()
                ps_pj = inner2[3].__enter__()
                kT = {}
                qT = {}
                expS = {}

                def dma_weights(c):
                    wq_t = wqk.tile([P, CB, P], BF16, tag="wq", name=f"wq{c}")
                    wk_t = wqk.tile([P, CB, P], BF16, tag="wk", name=f"wk{c}")
                    nc.gpsimd.dma_start(
                        wq_t[:],
                        wqT.rearrange("(ko p) n -> p ko n", p=P)[
                            :, :, ts(c, P)
                        ],
                    )
                    nc.gpsimd.dma_start(
                        wk_t[:],
                        wkT.rearrange("(ko p) n -> p ko n", p=P)[
                            :, :, ts(c, P)
                        ],
                    )
                    return wq_t, wk_t

                # ---- phase V: value projection, all heads ----
                # wv staged in 4 idle expS slots; accumulators in sc slots
                wvr = wvT.rearrange("(ko p) n -> p ko n", p=P)
                xTr = xT.rearrange("(ko p) t -> p ko t", p=P)
                wv_q = []
                for i in range(4):
                    w_t = spool.tile([P, 2, C], BF16, tag="expS", name=f"wvq{i}")
                    nc.gpsimd.dma_start(w_t[:], wvr[:, 2 * i : 2 * i + 2, :])
                    if i == 0:
                        nc.gpsimd.dma_start(xTp[0][:], xTr[:, :, 0:512])
                    wv_q.append(w_t)
                for i in range(1, 4):
                    nc.gpsimd.dma_start(
                        xTp[i][:], xTr[:, :, ts(i, 512)]
                    )
                nc.gpsimd.dma_start(
                    xTq_sb[:], xTq.rearrange("(ko p) t -> p ko t", p=P)
                )
                nc.gpsimd.dma_start(
                    masks_sb[:], masks[:].rearrange("s t p q -> p s t q")
                )
                wtiles = {0: dma_weights(0)}
                for tb in range(NB):
                    acc = ps_sc.tile([P, 2, 512], F32, tag="sc", name=f"vacc{tb}")
                    for kb in range(CB):
                        for dch in range(2):
                            nc.tensor.matmul(
                                acc[:, dch, :],
                                xTp[tb // 4][:, kb, ts(tb % 4, P)],
                                wv_q[kb // 2][:, kb % 2, ts(dch, 512)],
                                start=(kb == 0),
                                stop=(kb == CB - 1),
                            )
                    for dch in range(2):
                        eng = nc.scalar.copy if dch == 0 else (
                            nc.vector.tensor_copy)
                        eng(
                            vg[:, tb, dch * 8 : (dch + 1) * 8, 0:DK],
                            acc[:, dch, :].rearrange("p (h e) -> p h e", e=DK),
                        )

                def qk_units(c):
                    """Generator of PE units for pair-c Q/K projection."""
                    wq_t, wk_t = wtiles[c]
                    qT[c] = kq.tile([P, SLOTS * P], BF16, tag="qT", name=f"qT{c}")
                    kT[c] = kq.tile([P, T], BF16, tag="kT", name=f"kT{c}")
                    for dst, w_t, srcs, nnch, bias in (
                        (qT[c], wq_t, [xTq_sb, xTq_sb], 2, bq_sb),
                        (kT[c], wk_t, xTp, 4, bk_sb),
                    ):
                        for nch in range(nnch):
                            if srcs is xTp:
                                s_t, soff = xTp[nch], 0
                            else:
                                s_t, soff = xTq_sb, nch
                            acc = ps_pj.tile([P, 512], F32, tag="pj")
                            for kb in range(CB):
                                def mm(kb=kb, acc=acc, w_t=w_t, s_t=s_t, soff=soff):
                                    nc.tensor.matmul(
                                        acc[:],
                                        w_t[:, kb, :],
                                        s_t[:, kb, ts(soff, 512)]
                                        if s_t is xTq_sb
                                        else s_t[:, kb, :],
                                        start=(kb == 0),
                                        stop=(kb == CB - 1),
                                    )
                                yield mm
                            def drain(acc=acc, dst=dst, nch=nch, bias=bias):
                                nc.vector.tensor_scalar_add(
                                    dst[:, ts(nch, 512)], acc[:], bias[:, c : c + 1]
                                )
                            yield drain

                def sc_groups(c):
                    """List of per-jb closures: scores (row-tiled) + exp + mask."""
                    expS[c] = [
                        spool.tile([P, 2, SLOTS * P], BF16, tag="expS",
                                   name=f"expS{c}_{j}")
                        for j in range(NB)
                    ]
                    groups = []
                    for jb in ILV:
                        def grp(jb=jb, c=c):
                            sm = jb // 2
                            for ci, (qa, qb) in enumerate(qchunks(jb)):
                                w = qb - qa
                                pss = ps_sc.tile([P, 2, 512], F32, tag="sc")
                                for h in range(2):
                                    nc.tensor.matmul(
                                        pss[:, h, 0:w],
                                        kT[c][h * DK : (h + 1) * DK, ts(jb, P)],
                                        qT[c][h * DK : (h + 1) * DK, qa:qb],
                                        start=True,
                                        stop=True,
                                    )
                                nc.scalar.activation(
                                    expS[c][jb][:, :, qa:qb],
                                    pss[:, :, 0:w],
                                    EXP,
                                    scale=float(SCALE),
                                )
                                if ci == 0:
                                    for h in range(2):
                                        nc.vector.tensor_mul(
                                            expS[c][jb][:, h, ts(sm, P)],
                                            expS[c][jb][:, h, ts(sm, P)],
                                            masks_sb[:, sm, jb % 2, :],
                                        )
                        groups.append(grp)
                    return groups

                def attnv_units(c):
                    """Generator of PE units for pair-c attnv + normalization."""
                    for half in range(2):
                        qlo, qhi = half * 512, half * 512 + 512
                        pso = ps_av.tile([DK + 1, 2, 512], F32, tag="av")
                        jbs = list(range(8)) if half == 0 else list(ILV)
                        for idx, jb in enumerate(jbs):
                            q0 = P * (jb // 2)
                            qa = max(q0, qlo)
                            for h in range(2):
                                def mm(jb=jb, h=h, qa=qa, pso=pso, idx=idx,
                                       c=c, qlo=qlo, qhi=qhi, last=(idx == len(jbs) - 1)):
                                    nc.tensor.matmul(
                                        pso[:, h, qa - qlo : qhi - qlo],
                                        v[:, jb, 2 * c + h, :],
                                        expS[c][jb][:, h, qa:qhi],
                                        start=(idx == 0),
                                        stop=last,
                                    )
                                yield mm
                        def drain(pso=pso, half=half, qlo=qlo, c=c):
                            # stage O^T per-head to partitions 0:64/64:128 and
                            # the denom row to sbuf, releasing the psum banks
                            osb = osbp.tile([P, 512], BF16, tag="osb")
                            den = dnp.tile([1, 2, 512], BF16, tag="den")
                            nc.scalar.copy(den[:], pso[DK : DK + 1, :, :])
                            nc.vector.tensor_copy(osb[0:DK, :], pso[0:DK, 0, :])
                            nc.scalar.copy(osb[DK:P, :], pso[0:DK, 1, :])
                            # broadcast denoms (rows 0:64 <- head0, 64:128 <- head1)
                            psr = ps_r.tile([P, 512], F32, tag="r")
                            for h in range(2):
                                nc.tensor.matmul(
                                    psr[:],
                                    hm2_sb[0:1, h, :],
                                    den[0:1, h, :],
                                    start=(h == 0),
                                    stop=(h == 1),
                                )
                            r_sb = rsbp.tile([P, 512], F32, tag="rsb")
                            nc.vector.reciprocal_approx_fast(r_sb[:], psr[:])
                            for h in range(2):
                                nc.vector.tensor_mul(
                                    aT[c][h * DK : (h + 1) * DK, qlo : qlo + 512],
                                    osb[h * DK : (h + 1) * DK, :],
                                    r_sb[h * DK : (h + 1) * DK, :],
                                )
                        yield drain

                # ---- pipeline schedule ----
                POPS_AV = [7, 7, 6, 6, 5, 5, 5, 5, 2, 2, 2, 2, 1, 1, 0, 0]
                POPS_QK = [4, 4, 4, 4, 4, 4, 3, 3, 3, 3, 3, 3, 2, 2, 2, 2]

                # qk(7) holds back ~12 units that carry into tick 9
                # (otherwise PE-starved: only sc(7) + attnv(6) there)
                POPS_QK_HOLD = [4, 4, 4, 4, 3, 3, 3, 3, 3, 3, 2, 2, 2, 1, 1, 0]
                POPS_QK_TAIL = [3, 2, 2, 1, 1, 1, 1, 1, 0, 0, 0, 0, 0, 0, 0, 0]

                def tick(sc_c, qg, ag, pops_qk, drain_qk=True):
                    """One pipeline tick: interleave pair-sc_c scores with
                    projection and attnv units on the PE queue. attnv pops are
                    front-loaded so expS slot releases always precede the exp
                    writes that reuse them (no sem cycle)."""
                    sgs = sc_groups(sc_c)
                    for g in range(NB):
                        sgs[g]()
                        _pop(qg, pops_qk[g])
                        _pop(ag, POPS_AV[g])
                    if drain_qk:
                        _pop(qg, 99)
                    _pop(ag, 99)

                EMPTY = iter(())
                # tick 1: QK(0) dense; prefetch w(1)
                for u in qk_units(0):
                    u()
                wtiles[1] = dma_weights(1)
                # tick 2: QK(1) x sc(0)
                tick(0, iter(qk_units(1)), EMPTY, POPS_QK)
                wtiles[2] = dma_weights(2)
                # ticks 3..7: QK(c+2) x sc(c+1) x attnv(c)
                for c in range(5):
                    tick(c + 1, iter(qk_units(c + 2)), iter(attnv_units(c)),
                         POPS_QK)
                    if c + 3 < CB:
                        wtiles[c + 3] = dma_weights(c + 3)
                # tick 8: QK(7) partially held back
                qg7 = iter(qk_units(7))
                tick(6, qg7, iter(attnv_units(5)), POPS_QK_HOLD, drain_qk=False)
                # tick 9: sc(7) x attnv(6) x leftover QK(7); then attnv(7)
                # half0 (its exps are done by group 14)
                ag7 = iter(attnv_units(7))
                tick(7, qg7, iter(attnv_units(6)), POPS_QK_TAIL)
                _pop(ag7, 17)   # half0 MMs + norm drain
                for pool in reversed(inner2):
                    pool.__exit__(None, None, None)

                # ---- output projection, overlapped with attnv(7) ----
                with (
                    tc.tile_pool(name="out", bufs=1) as opool,
                    tc.tile_pool(name="yt", bufs=2) as ytp,
                    tc.tile_pool(name="ps_y", bufs=2, space="PSUM") as ps_y,
                ):
                    wor = woT.rearrange("(ko p) n -> p ko n", p=P)
                    woT_h = []
                    for nch in range(2):
                        w_t = opool.tile([P, CB, 512], BF16, tag=f"wo{nch}",
                                         name=f"wo{nch}")
                        nc.gpsimd.dma_start(w_t[:], wor[:, :, ts(nch, 512)])
                        woT_h.append(w_t)

                    def o_units():
                        yr = y.rearrange("(tb p) c -> p tb c", p=P)
                        for tb in range(SLOTS):
                            for nch in range(2):
                                y_t = ytp.tile([P, 512], F32, tag="yt",
                                               name=f"y{tb}_{nch}")
                                psy = ps_y.tile([P, 512], F32, tag="ps_y")
                                for cbk in range(CB):
                                    def mm(psy=psy, cbk=cbk, tb=tb, nch=nch):
                                        nc.tensor.matmul(
                                            psy[:],
                                            aT[cbk][:, ts(tb, P)],
                                            woT_h[nch][:, cbk, :],
                                            start=(cbk == 0),
                                            stop=(cbk == CB - 1),
                                        )
                                    yield mm
                                def drain(psy=psy, y_t=y_t, tb=tb, nch=nch):
                                    nc.vector.tensor_copy(y_t[:], psy[:])
                                    nc.gpsimd.dma_start(
                                        yr[:, tb, ts(nch, 512)], y_t[:]
                                    )
                                yield drain

                    og = iter(o_units())
                    for _ in range(17):
                        _pop(ag7, 2)
                        _pop(og, 2)
                    _pop(og, 999)

    nc.compile()
    return nc


def _host_inputs(x, mask, Wq, bq_v, Wk, bk_v, Wv, bv_v, Wo, bo_v):
    """Per-core input maps + the host-side output bias correction."""
    f32 = np.float32
    bf16 = ml_dtypes.bfloat16
    wqT = np.ascontiguousarray(np.asarray(Wq, f32).T).astype(bf16)
    wkT = np.ascontiguousarray(np.asarray(Wk, f32).T).astype(bf16)
    wvT = np.ascontiguousarray(np.asarray(Wv, f32).T).astype(bf16)
    woT = np.ascontiguousarray(np.asarray(Wo, f32).T).astype(bf16)
    bq_p = np.ascontiguousarray(np.asarray(bq_v, f32).reshape(C // P, P).T)
    bk_p = np.ascontiguousarray(np.asarray(bk_v, f32).reshape(C // P, P).T)
    # exact v/o bias fold: softmax rows sum to 1, so v+bv adds bv to attn out
    bo_eff = (np.asarray(bo_v, f32) + np.asarray(bv_v, f32) @ np.asarray(Wo, f32).T)
    # hm2: rank-2 broadcast matrix for per-head reciprocal rows
    hm2_np = np.zeros((1, 2, P), f32)
    hm2_np[0, 0, 0:DK] = 1.0
    hm2_np[0, 1, DK:P] = 1.0
    hm2_np = hm2_np.astype(bf16)

    # per-half causal boundary masks for the last two key blocks of each slot
    mask_half = []
    tri = np.tril(np.ones((P, P), f32)).T  # [j, i] = 1 where j <= i
    for half in range(2):
        m = np.zeros((SLOTS, 2, P, P), f32)
        for s in range(SLOTS):
            g = QBLKS[half][s]
            for idx, jb in enumerate((2 * s, 2 * s + 1)):
                if jb < g:
                    m[s, idx] = 1.0
                elif jb == g:
                    m[s, idx] = tri
        mask_half.append(m.astype(bf16))

    xn = np.asarray(x, f32)
    in_maps = []
    for core in range(8):
        b, half = divmod(core, 2)
        xT = np.ascontiguousarray(xn[b].T).astype(bf16)
        qtok = np.concatenate([np.arange(g * P, (g + 1) * P) for g in QBLKS[half]])
        xTq = np.ascontiguousarray(xn[b][qtok].T).astype(bf16)
        in_maps.append(
            {
                "xT": xT,
                "xTq": xTq,
                "wqT": wqT,
                "wkT": wkT,
                "wvT": wvT,
                "woT": woT,
                "bq": bq_p,
                "bk": bk_p,
                "masks": mask_half[half],
                "hm2": hm2_np,
            }
        )
    return in_maps, bo_eff


def _run(inputs, trace=False):
    if "nc" not in _cache:
        _cache["nc"] = _build()
    nc = _cache["nc"]
    in_maps, bo_eff = _host_inputs(
        inputs["x"], inputs["mask"],
        inputs["Wq"], inputs["bq"], inputs["Wk"], inputs["bk"],
        inputs["Wv"], inputs["bv"], inputs["Wo"], inputs["bo"],
    )
    res = run_bass_kernel_spmd(nc, in_maps, list(range(8)), trace=trace)
    out = np.empty((B, T, C), np.float32)
    for core in range(8):
        b, half = divmod(core, 2)
        yc = res.results[core]["y"]
        for s, g in enumerate(QBLKS[half]):
            out[b, g * P : (g + 1) * P] = yc[s * P : (s + 1) * P]
    out += bo_eff
    return out, res


def kernel(**inputs):
    out, _ = _run(inputs, trace=False)
    return out


# revision 9
# speedup vs baseline: 1.0391x; 1.0165x over previous
"""Multi-head causal attention (B=4, T=2048, C=1024, H=16) on 8 trn2 cores.

Sharding: data-parallel over batch (4) x sequence-parallel over causal query
blocks (2), zig-zag balanced: core = 2*b + half; half 0 gets query blocks
[0,2,4,6,9,11,13,15], half 1 gets [1,3,5,7,8,10,12,14]. Slot s processes
key blocks 0..2s+1; causal boundary via per-core input masks.

Kernel structure (v2): software-pipelined over head pairs c=0..7.
  tick: [Q/K proj pair c+1] x [scores+exp+mask pair c] x [attnv pair c-1]
Scores are row-tiled (K=64, two heads in PE row strips 0/64 concurrently).
attnv is v-stationary (v tile [128,65] incl ones column -> softmax denom in
psum row 64), streaming wide query chunks; output lands as O^T = aT layout
(no transpose phase). Normalization: reciprocal of denom row + rank-2
broadcast matmul (hm2) + partition-shifted DVE multiplies into aT.
"""

import numpy as np
import ml_dtypes

import concourse.bass as bass
import concourse.mybir as mybir
import concourse.tile as tile
from concourse import bacc
from concourse.bass import ts
from concourse.bass_utils import run_bass_kernel_spmd

B, T, C, H, DK = 4, 2048, 1024, 16, 64
P = 128
NB = T // P          # 16 key blocks
SLOTS = 8            # query blocks per core
CB = C // P          # 8 channel blocks = head pairs
SCALE = 1.0 / np.sqrt(DK)
BF16 = mybir.dt.bfloat16
F32 = mybir.dt.float32
EXP = mybir.ActivationFunctionType.Exp

QBLKS = [
    [0, 2, 4, 6, 9, 11, 13, 15],
    [1, 3, 5, 7, 8, 10, 12, 14],
]

# interleaved jb order: evens ACT load per group and matches attnv's
# half-1 read order so expS slot releases track allocations
ILV = [0, 8, 1, 9, 2, 10, 3, 11, 4, 12, 5, 13, 6, 14, 7, 15]

_cache = {}


def _pop(gen, n):
    for _ in range(n):
        try:
            next(gen)()
        except StopIteration:
            return


def _build():
    nc = bacc.Bacc("TRN2", target_bir_lowering=False, debug=False)

    xT = nc.dram_tensor("xT", [C, T], BF16, kind="ExternalInput").ap()
    xTq = nc.dram_tensor("xTq", [C, SLOTS * P], BF16, kind="ExternalInput").ap()
    wqT = nc.dram_tensor("wqT", [C, C], BF16, kind="ExternalInput").ap()
    wkT = nc.dram_tensor("wkT", [C, C], BF16, kind="ExternalInput").ap()
    wvT = nc.dram_tensor("wvT", [C, C], BF16, kind="ExternalInput").ap()
    woT = nc.dram_tensor("woT", [C, C], BF16, kind="ExternalInput").ap()
    bq = nc.dram_tensor("bq", [P, CB], F32, kind="ExternalInput").ap()
    bk = nc.dram_tensor("bk", [P, CB], F32, kind="ExternalInput").ap()
    masks = nc.dram_tensor("masks", [SLOTS, 2, P, P], BF16, kind="ExternalInput").ap()
    hm2 = nc.dram_tensor("hm2", [1, 2, P], BF16, kind="ExternalInput").ap()
    y = nc.dram_tensor("y", [SLOTS * P, C], F32, kind="ExternalOutput").ap()

    def qchunks(jb):
        """score q-chunks for key block jb: [q0,512) and/or [512,1024)."""
        q0 = P * (jb // 2)
        out = []
        if q0 < 512:
            out.append((q0, 512))
            out.append((512, 1024))
        else:
            out.append((q0, 1024))
        return out

    with tile.TileContext(nc) as tc:
        with (
            tc.tile_pool(name="const", bufs=1) as cpool,
            tc.tile_pool(name="big", bufs=1) as bigpool,
        ):
            hm2_sb = cpool.tile([1, 2, P], BF16)

            v = bigpool.tile([P, NB, H, DK + 1], BF16)
            aT = {
                c: bigpool.tile([P, SLOTS * P], BF16, tag=f"aT{c}",
                                name=f"aT{c}")
                for c in range(CB)
            }
            vg = v[:]
            nc.vector.memset(vg[:, :, :, DK : DK + 1], 1.0)

            # ---- outer loop pools (survive into the O phase) ----
            with (
                tc.tile_pool(name="expS", bufs=18) as spool,
                tc.tile_pool(name="osb", bufs=2) as osbp,
                tc.tile_pool(name="dnp", bufs=1) as dnp,
                tc.tile_pool(name="rsb", bufs=1) as rsbp,
                tc.tile_pool(name="ps_av", bufs=1, space="PSUM") as ps_av,
                tc.tile_pool(name="ps_r", bufs=1, space="PSUM") as ps_r,
                tc.tile_pool(name="xbig", bufs=1) as xbig,
            ):
                masks_sb = xbig.tile([P, SLOTS, 2, P], BF16)
                bq_sb = xbig.tile([P, CB], F32)
                bk_sb = xbig.tile([P, CB], F32)
                xTp = [
                    xbig.tile([P, CB, 512], BF16, tag=f"xTp{i}", name=f"xTp{i}")
                    for i in range(4)
                ]
                xTq_sb = xbig.tile([P, CB, SLOTS * P], BF16)


                # ---- main loop pools (entered early; phase V borrows
                # their slots, so no pool-close barrier gates tick 1) ----
                inner2 = [
                    tc.tile_pool(name="wqk", bufs=2),
                    tc.tile_pool(name="kq", bufs=2),
                    tc.tile_pool(name="ps_sc", bufs=2, space="PSUM"),
                    tc.tile_pool(name="ps_pj", bufs=1, space="PSUM"),
                ]
                wqk = inner2[0].__enter__()
                kq = inner2[1].__enter__()
                ps_sc = inner2[2].__enter__()
                ps_pj = inner2[3].__enter__()
                kT = {}
                qT = {}
                expS = {}

                def dma_weights(c):
                    wq_t = wqk.tile([P, CB, P], BF16, tag="wq", name=f"wq{c}")
                    wk_t = wqk.tile([P, CB, P], BF16, tag="wk", name=f"wk{c}")
                    nc.gpsimd.dma_start(
                        wq_t[:],
                        wqT.rearrange("(ko p) n -> p ko n", p=P)[
                            :, :, ts(c, P)
                        ],
                    )
                    nc.gpsimd.dma_start(
                        wk_t[:],
                        wkT.rearrange("(ko p) n -> p ko n", p=P)[
                            :, :, ts(c, P)
                        ],
                    )
                    return wq_t, wk_t

                # ---- phase V: value projection, all heads ----
                # wv staged in 4 idle expS slots; accumulators in sc slots
                wvr = wvT.rearrange("(ko p) n -> p ko n", p=P)
                xTr = xT.rearrange("(ko p) t -> p ko t", p=P)
                wv_q = []
                for i in range(4):
                    w_t = spool.tile([P, 2, C], BF16, tag="expS", name=f"wvq{i}")
                    nc.gpsimd.dma_start(w_t[:], wvr[:, 2 * i : 2 * i + 2, :])
                    if i == 0:
                        nc.gpsimd.dma_start(xTp[0][:], xTr[:, :, 0:512])
                    wv_q.append(w_t)
                for i in range(1, 4):
                    nc.gpsimd.dma_start(
                        xTp[i][:], xTr[:, :, ts(i, 512)]
                    )
                nc.gpsimd.dma_start(
                    xTq_sb[:], xTq.rearrange("(ko p) t -> p ko t", p=P)
                )
                nc.gpsimd.dma_start(bq_sb[:], bq[:])
                nc.gpsimd.dma_start(bk_sb[:], bk[:])
                nc.gpsimd.dma_start(hm2_sb[:], hm2[:])
                wtiles = {0: dma_weights(0)}
                nc.gpsimd.dma_start(
                    masks_sb[:], masks[:].rearrange("s t p q -> p s t q")
                )
                for tb in range(NB):
                    acc = ps_sc.tile([P, 2, 512], F32, tag="sc", name=f"vacc{tb}")
                    for kb in range(CB):
                        for dch in range(2):
                            nc.tensor.matmul(
                                acc[:, dch, :],
                                xTp[tb // 4][:, kb, ts(tb % 4, P)],
                                wv_q[kb // 2][:, kb % 2, ts(dch, 512)],
                                start=(kb == 0),
                                stop=(kb == CB - 1),
                            )
                    for dch in range(2):
                        eng = nc.scalar.copy if dch == 0 else (
                            nc.vector.tensor_copy)
                        eng(
                            vg[:, tb, dch * 8 : (dch + 1) * 8, 0:DK],
                            acc[:, dch, :].rearrange("p (h e) -> p h e", e=DK),
                        )

                def qk_units(c):
                    """Generator of PE units for pair-c Q/K projection."""
                    wq_t, wk_t = wtiles[c]
                    qT[c] = kq.tile([P, SLOTS * P], BF16, tag="qT", name=f"qT{c}")
                    kT[c] = kq.tile([P, T], BF16, tag="kT", name=f"kT{c}")
                    for dst, w_t, srcs, nnch, bias in (
                        (qT[c], wq_t, [xTq_sb, xTq_sb], 2, bq_sb),
                        (kT[c], wk_t, xTp, 4, bk_sb),
                    ):
                        for nch in range(nnch):
                            if srcs is xTp:
                                s_t, soff = xTp[nch], 0
                            else:
                                s_t, soff = xTq_sb, nch
                            acc = ps_pj.tile([P, 512], F32, tag="pj")
                            for kb in range(CB):
                                def mm(kb=kb, acc=acc, w_t=w_t, s_t=s_t, soff=soff):
                                    nc.tensor.matmul(
                                        acc[:],
                                        w_t[:, kb, :],
                                        s_t[:, kb, ts(soff, 512)]
                                        if s_t is xTq_sb
                                        else s_t[:, kb, :],
                                        start=(kb == 0),
                                        stop=(kb == CB - 1),
                                    )
                                yield mm
                            def drain(acc=acc, dst=dst, nch=nch, bias=bias):
                                nc.vector.tensor_scalar_add(
                                    dst[:, ts(nch, 512)], acc[:], bias[:, c : c + 1]
                                )
                            yield drain

                def sc_groups(c):
                    """List of per-jb closures: scores (row-tiled) + exp + mask."""
                    expS[c] = [
                        spool.tile([P, 2, SLOTS * P], BF16, tag="expS",
                                   name=f"expS{c}_{j}")
                        for j in range(NB)
                    ]
                    groups = []
                    for jb in ILV:
                        def grp(jb=jb, c=c):
                            sm = jb // 2
                            for ci, (qa, qb) in enumerate(qchunks(jb)):
                                w = qb - qa
                                pss = ps_sc.tile([P, 2, 512], F32, tag="sc")
                                for h in range(2):
                                    nc.tensor.matmul(
                                        pss[:, h, 0:w],
                                        kT[c][h * DK : (h + 1) * DK, ts(jb, P)],
                                        qT[c][h * DK : (h + 1) * DK, qa:qb],
                                        start=True,
                                        stop=True,
                                    )
                                nc.scalar.activation(
                                    expS[c][jb][:, :, qa:qb],
                                    pss[:, :, 0:w],
                                    EXP,
                                    scale=float(SCALE),
                                )
                                if ci == 0:
                                    for h in range(2):
                                        nc.vector.tensor_mul(
                                            expS[c][jb][:, h, ts(sm, P)],
                                            expS[c][jb][:, h, ts(sm, P)],
                                            masks_sb[:, sm, jb % 2, :],
                                        )
                        groups.append(grp)
                    return groups

                def attnv_units(c):
                    """Generator of PE units for pair-c attnv + normalization."""
                    for half in range(2):
                        qlo, qhi = half * 512, half * 512 + 512
                        pso = ps_av.tile([DK + 1, 2, 512], F32, tag="av")
                        jbs = list(range(8)) if half == 0 else list(ILV)
                        for idx, jb in enumerate(jbs):
                            q0 = P * (jb // 2)
                            qa = max(q0, qlo)
                            for h in range(2):
                                def mm(jb=jb, h=h, qa=qa, pso=pso, idx=idx,
                                       c=c, qlo=qlo, qhi=qhi, last=(idx == len(jbs) - 1)):
                                    nc.tensor.matmul(
                                        pso[:, h, qa - qlo : qhi - qlo],
                                        v[:, jb, 2 * c + h, :],
                                        expS[c][jb][:, h, qa:qhi],
                                        start=(idx == 0),
                                        stop=last,
                                    )
                                yield mm
                        def drain(pso=pso, half=half, qlo=qlo, c=c):
                            # stage O^T per-head to partitions 0:64/64:128 and
                            # the denom row to sbuf, releasing the psum banks
                            osb = osbp.tile([P, 512], BF16, tag="osb")
                            den = dnp.tile([1, 2, 512], BF16, tag="den")
                            nc.scalar.copy(den[:], pso[DK : DK + 1, :, :])
                            nc.vector.tensor_copy(osb[0:DK, :], pso[0:DK, 0, :])
                            nc.scalar.copy(osb[DK:P, :], pso[0:DK, 1, :])
                            # broadcast denoms (rows 0:64 <- head0, 64:128 <- head1)
                            psr = ps_r.tile([P, 512], F32, tag="r")
                            for h in range(2):
                                nc.tensor.matmul(
                                    psr[:],
                                    hm2_sb[0:1, h, :],
                                    den[0:1, h, :],
                                    start=(h == 0),
                                    stop=(h == 1),
                                )
                            r_sb = rsbp.tile([P, 512], F32, tag="rsb")
                            nc.vector.reciprocal_approx_fast(r_sb[:], psr[:])
                            for h in range(2):
                                nc.vector.tensor_mul(
                                    aT[c][h * DK : (h + 1) * DK, qlo : qlo + 512],
                                    osb[h * DK : (h + 1) * DK, :],
                                    r_sb[h * DK : (h + 1) * DK, :],
                                )
                        yield drain

                # ---- pipeline schedule ----
                POPS_AV = [7, 7, 6, 6, 5, 5, 5, 5, 2, 2, 2, 2, 1, 1, 0, 0]
                POPS_QK = [4, 4, 4, 4, 4, 4, 3, 3, 3, 3, 3, 3, 2, 2, 2, 2]

                # qk(7) holds back ~12 units that carry into tick 9
                # (otherwise PE-starved: only sc(7) + attnv(6) there)
                POPS_QK_HOLD = [4, 4, 4, 4, 3, 3, 3, 3, 3, 3, 2, 2, 2, 1, 1, 0]
                POPS_QK_TAIL = [3, 2, 2, 1, 1, 1, 1, 1, 0, 0, 0, 0, 0, 0, 0, 0]

                def tick(sc_c, qg, ag, pops_qk, drain_qk=True):
                    """One pipeline tick: interleave pair-sc_c scores with
                    projection and attnv units on the PE queue. attnv pops are
                    front-loaded so expS slot releases always precede the exp
                    writes that reuse them (no sem cycle)."""
                    sgs = sc_groups(sc_c)
                    for g in range(NB):
                        sgs[g]()
                        _pop(qg, pops_qk[g])
                        _pop(ag, POPS_AV[g])
                    if drain_qk:
                        _pop(qg, 99)
                    _pop(ag, 99)

                EMPTY = iter(())
                # tick 1: QK(0) dense; prefetch w(1)
                for u in qk_units(0):
                    u()
                wtiles[1] = dma_weights(1)
                # tick 2: QK(1) x sc(0)
                tick(0, iter(qk_units(1)), EMPTY, POPS_QK)
                wtiles[2] = dma_weights(2)
                # ticks 3..7: QK(c+2) x sc(c+1) x attnv(c)
                for c in range(5):
                    tick(c + 1, iter(qk_units(c + 2)), iter(attnv_units(c)),
                         POPS_QK)
                    if c + 3 < CB:
                        wtiles[c + 3] = dma_weights(c + 3)
                # tick 8: QK(7) partially held back
                qg7 = iter(qk_units(7))
                tick(6, qg7, iter(attnv_units(5)), POPS_QK_HOLD, drain_qk=False)
                # tick 9: sc(7) x attnv(6) x leftover QK(7); then attnv(7)
                # half0 (its exps are done by group 14)
                ag7 = iter(attnv_units(7))
                tick(7, qg7, iter(attnv_units(6)), POPS_QK_TAIL)
                _pop(ag7, 17)   # half0 MMs + norm drain
                for pool in reversed(inner2):
                    pool.__exit__(None, None, None)

                # ---- output projection, overlapped with attnv(7) ----
                with (
                    tc.tile_pool(name="out", bufs=1) as opool,
                    tc.tile_pool(name="yt", bufs=2) as ytp,
                    tc.tile_pool(name="ps_y", bufs=2, space="PSUM") as ps_y,
                ):
                    wor = woT.rearrange("(ko p) n -> p ko n", p=P)
                    woT_h = []
                    for nch in range(2):
                        w_t = opool.tile([P, CB, 512], BF16, tag=f"wo{nch}",
                                         name=f"wo{nch}")
                        nc.gpsimd.dma_start(w_t[:], wor[:, :, ts(nch, 512)])
                        woT_h.append(w_t)

                    def o_units():
                        yr = y.rearrange("(tb p) c -> p tb c", p=P)
                        for tb in range(SLOTS):
                            for nch in range(2):
                                y_t = ytp.tile([P, 512], F32, tag="yt",
                                               name=f"y{tb}_{nch}")
                                psy = ps_y.tile([P, 512], F32, tag="ps_y")
                                for cbk in range(CB):
                                    def mm(psy=psy, cbk=cbk, tb=tb, nch=nch):
                                        nc.tensor.matmul(
                                            psy[:],
                                            aT[cbk][:, ts(tb, P)],
                                            woT_h[nch][:, cbk, :],
                                            start=(cbk == 0),
                                            stop=(cbk == CB - 1),
                                        )
                                    yield mm
                                def drain(psy=psy, y_t=y_t, tb=tb, nch=nch):
                                    nc.vector.tensor_copy(y_t[:], psy[:])
                                    nc.gpsimd.dma_start(
                                        yr[:, tb, ts(nch, 512)], y_t[:]
                                    )
                                yield drain

                    og = iter(o_units())
                    for _ in range(17):
                        _pop(ag7, 2)
                        _pop(og, 2)
                    _pop(og, 999)

    nc.compile()
    return nc


def _host_inputs(x, mask, Wq, bq_v, Wk, bk_v, Wv, bv_v, Wo, bo_v):
    """Per-core input maps + the host-side output bias correction."""
    f32 = np.float32
    bf16 = ml_dtypes.bfloat16
    wqT = np.ascontiguousarray(np.asarray(Wq, f32).T).astype(bf16)
    wkT = np.ascontiguousarray(np.asarray(Wk, f32).T).astype(bf16)
    wvT = np.ascontiguousarray(np.asarray(Wv, f32).T).astype(bf16)
    woT = np.ascontiguousarray(np.asarray(Wo, f32).T).astype(bf16)
    bq_p = np.ascontiguousarray(np.asarray(bq_v, f32).reshape(C // P, P).T)
    bk_p = np.ascontiguousarray(np.asarray(bk_v, f32).reshape(C // P, P).T)
    # exact v/o bias fold: softmax rows sum to 1, so v+bv adds bv to attn out
    bo_eff = (np.asarray(bo_v, f32) + np.asarray(bv_v, f32) @ np.asarray(Wo, f32).T)
    # hm2: rank-2 broadcast matrix for per-head reciprocal rows
    hm2_np = np.zeros((1, 2, P), f32)
    hm2_np[0, 0, 0:DK] = 1.0
    hm2_np[0, 1, DK:P] = 1.0
    hm2_np = hm2_np.astype(bf16)

    # per-half causal boundary masks for the last two key blocks of each slot
    mask_half = []
    tri = np.tril(np.ones((P, P), f32)).T  # [j, i] = 1 where j <= i
    for half in range(2):
        m = np.zeros((SLOTS, 2, P, P), f32)
        for s in range(SLOTS):
            g = QBLKS[half][s]
            for idx, jb in enumerate((2 * s, 2 * s + 1)):
                if jb < g:
                    m[s, idx] = 1.0
                elif jb == g:
                    m[s, idx] = tri
        mask_half.append(m.astype(bf16))

    xn = np.asarray(x, f32)
    in_maps = []
    for core in range(8):
        b, half = divmod(core, 2)
        xT = np.ascontiguousarray(xn[b].T).astype(bf16)
        qtok = np.concatenate([np.arange(g * P, (g + 1) * P) for g in QBLKS[half]])
        xTq = np.ascontiguousarray(xn[b][qtok].T).astype(bf16)
        in_maps.append(
            {
                "xT": xT,
                "xTq": xTq,
                "wqT": wqT,
                "wkT": wkT,
                "wvT": wvT,
                "woT": woT,
                "bq": bq_p,
                "bk": bk_p,
                "masks": mask_half[half],
                "hm2": hm2_np,
            }
        )
    return in_maps, bo_eff


def _run(inputs, trace=False):
    if "nc" not in _cache:
        _cache["nc"] = _build()
    nc = _cache["nc"]
    in_maps, bo_eff = _host_inputs(
        inputs["x"], inputs["mask"],
        inputs["Wq"], inputs["bq"], inputs["Wk"], inputs["bk"],
        inputs["Wv"], inputs["bv"], inputs["Wo"], inputs["bo"],
    )
    res = run_bass_kernel_spmd(nc, in_maps, list(range(8)), trace=trace)
    out = np.empty((B, T, C), np.float32)
    for core in range(8):
        b, half = divmod(core, 2)
        yc = res.results[core]["y"]
        for s, g in enumerate(QBLKS[half]):
            out[b, g * P : (g + 1) * P] = yc[s * P : (s + 1) * P]
    out += bo_eff
    return out, res


def kernel(**inputs):
    out, _ = _run(inputs, trace=False)
    return out
